# revision 40
# baseline (speedup 1.0000x reference)
"""AdaptiveSamplingMixing — Trainium2 SPMD kernel (4 cores, group-parallel).

Measurement reality on this setup: the axon tunnel moves host<->device data at
~50 MB/s while the device kernel itself executes in ~2 ms/core, so the
end-to-end run_bass_kernel_spmd call is dominated by input upload.  The kernel
is organised to minimise uploaded bytes (123 MB baseline -> ~58 MB):

  - core = g (one sampling group per core, all 4 images).  The previous
    (image-pair x group) split duplicated pg_w/op_w across image pairs
    (+33 MB); group-sharding uploads every weight byte exactly once.
  - features, pg_w and op_w upload as absmax-scaled 9-bit fixed point
    (8 values in 9 u8 planes); cw and qft as 10-bit (4 values in 5 planes).
    The vector engine decodes (shift/and/scale) on device.  The feature
    descale folds into the bilinear weights, pg_w's into the params-GEMM
    psum evacuation, op_w's into the host-side group sum, so dequantisation
    costs no extra device passes.  End-to-end error ~1.1e-2 (vs ~5.2e-3 for
    all-bf16) against the 2e-2 gate; the numpy emulation of this pipeline
    predicts hardware error to ~1e-4, so the bit widths were chosen from a
    measured error/bytes sweep (fp8 fails at 2-5e-2: coherent-sum effect).
    Touched-pixel feature compaction was evaluated and shelved: the touched
    fraction swings 27%..75% across same-distribution PRNG draws, so a safe
    capacity erases the saving.
  - bilinear x-corner PAIRS are fetched with a single 512 B dma_gather
    descriptor via an overlapping strided view of the f32 feature buffer
    (elem_step=64 elems = 256 B, elem_size=128), halving gather descriptors
    and halving the uploaded index tables.
  - the output projection contracts over the POUT partition axis with PSUM
    accumulation over d, keeping h2 in SBUF (no DRAM round-trip / transposing
    regather).
  - partial outputs download as bf16.
  - kernel() runs the spmd call twice (first call warms the jax/PJRT
    executable cache); the steady-state call is what a benchmark observes.

Numerics: rsqrt factors of both inner layernorms are folded out algebraically
(LN2(r*X) == LN2(X) for per-query scales); mix2's r2 is applied as a per-query
column scale after the projection matmul.  The host computes addressing
metadata (sample indices / bilinear weights), packs/reshapes inputs, and
finishes with the 4-way group partial-sum + residual + final affine LayerNorm.
"""
import sys
sys.path.insert(0, "/opt/trn_rl_repo")
import numpy as np
import ml_dtypes

import concourse.bass as bass
import concourse.mybir as mybir
import concourse.tile as tile
from concourse import bacc
from concourse.ap import AP
from concourse.bass_utils import run_bass_kernel_spmd
from concourse.masks import make_identity

F32 = mybir.dt.float32
BF16 = mybir.dt.bfloat16
I16 = mybir.dt.int16
U8 = mybir.dt.uint8
AL = mybir.AluOpType
AF = mybir.ActivationFunctionType
AX = mybir.AxisListType
BF = ml_dtypes.bfloat16

B, N, D = 4, 300, 256
G, PIN, POUT = 4, 32, 128
CG = D // G  # 64
STRIDES = (8, 16, 32, 64)
SIZES = ((100, 160), (50, 80), (25, 40), (13, 20))
TAU, MAP_STRIDE = 2.0, 3.0

QI = 300                   # queries per image
QT = B * QI                # 1200 queries per core (all images)
PIX_IMG = sum(h * w for h, w in SIZES)           # 21260 pixels per image
LVL_BASE = (0, 16000, 20000, 21000)
LVL_ROWS = (16000, 4000, 1000, 260)
PIXPAD = 128 * 665         # 85120 >= 1 + 4*21260 + 1 (lead/tail pads)
FLATC = PIXPAD * CG // 128                       # 42560 flat cols of fb
NSMP = QI * PIN            # 9600 gather indices per (img, lvl, ycorner)
NGRP = 32                  # gather groups per core: img(4) x lvl(4) x y(2)

_CACHE = {}


def _pack10(x, plane_axis):
    """absmax-scaled signed 10-bit fixed point; 4 elems -> 5 u8 planes.

    Packs along the last axis (length divisible by 4).  Returns (planes u8
    with a new length-5 axis at plane_axis, scale s) where
    dequant = (int10 + 512 offset removed) / s.
    """
    x = np.asarray(x, np.float32)
    amax = float(np.abs(x).max())
    if not np.isfinite(amax) or amax == 0.0:
        amax = 1.0
    s = 511.0 / amax
    q = (np.clip(np.round(x * s), -512, 511).astype(np.int16) + 512).astype(
        np.uint16)
    q = q.reshape(*x.shape[:-1], x.shape[-1] // 4, 4)
    L = (q & 255).astype(np.uint8)
    H = ((q[..., 0] >> 8) | ((q[..., 1] >> 8) << 2) |
         ((q[..., 2] >> 8) << 4) | ((q[..., 3] >> 8) << 6)).astype(np.uint8)
    planes = np.stack([L[..., 0], L[..., 1], L[..., 2], L[..., 3], H],
                      axis=plane_axis)
    return np.ascontiguousarray(planes), s


def _pack9(x, plane_axis):
    """absmax-scaled signed 9-bit fixed point; 8 elems -> 9 u8 planes."""
    x = np.asarray(x, np.float32)
    amax = float(np.abs(x).max())
    if not np.isfinite(amax) or amax == 0.0:
        amax = 1.0
    s = 255.0 / amax
    q = (np.clip(np.round(x * s), -256, 255).astype(np.int16) + 256).astype(
        np.uint16)
    q = q.reshape(*x.shape[:-1], x.shape[-1] // 8, 8)
    L = (q & 255).astype(np.uint8)
    H = np.zeros(q.shape[:-1], np.uint8)
    for i in range(8):
        H |= ((q[..., i] >> 8) << i).astype(np.uint8)
    planes = np.stack([L[..., i] for i in range(8)] + [H], axis=plane_axis)
    return np.ascontiguousarray(planes), s


def _dec9(nc, pool, planes, out3, c, tagp=""):
    """device decode: planes [128,9,c] u8 -> out3 [128,c,8] float view."""
    for i in range(8):
        hi8 = pool.tile([128, c], U8, tag=f"hi8{tagp}")
        if i == 0:
            nc.vector.tensor_scalar(hi8[:, :], planes[:, 8, :], 1, None,
                                    AL.bitwise_and)
        else:
            nc.vector.tensor_scalar(hi8[:, :], planes[:, 8, :], i, None,
                                    AL.logical_shift_right)
            nc.vector.tensor_scalar(hi8[:, :], hi8[:, :], 1, None,
                                    AL.bitwise_and)
        hif = pool.tile([128, c], F32, tag=f"hif{tagp}")
        nc.vector.tensor_copy(hif[:, :], hi8[:, :])
        lof = pool.tile([128, c], F32, tag=f"lof{tagp}")
        nc.vector.tensor_copy(lof[:, :], planes[:, i, :])
        nc.vector.tensor_scalar(hif[:, :], hif[:, :], 256.0, None, AL.mult)
        nc.vector.tensor_tensor(hif[:, :], hif[:, :], lof[:, :], AL.add)
        nc.vector.tensor_scalar(out3[:, :, i], hif[:, :], 256.0, None,
                                AL.subtract)


def _dec10(nc, pool, planes, out3, c, tagp=""):
    """device decode: planes [128,5,c] u8 -> out3 [128,c,4] (f32/bf16 view),
    values = int10 (offset removed), i.e. true*scale."""
    AL_ = AL
    for i in range(4):
        hi8 = pool.tile([128, c], U8, tag=f"hi8{tagp}")
        if i == 0:
            nc.vector.tensor_scalar(hi8[:, :], planes[:, 4, :], 3, None,
                                    AL_.bitwise_and)
        else:
            nc.vector.tensor_scalar(hi8[:, :], planes[:, 4, :], 2 * i,
                                    None, AL_.logical_shift_right)
            nc.vector.tensor_scalar(hi8[:, :], hi8[:, :], 3, None,
                                    AL_.bitwise_and)
        hif = pool.tile([128, c], F32, tag=f"hif{tagp}")
        nc.vector.tensor_copy(hif[:, :], hi8[:, :])
        lof = pool.tile([128, c], F32, tag=f"lof{tagp}")
        nc.vector.tensor_copy(lof[:, :], planes[:, i, :])
        nc.vector.tensor_scalar(hif[:, :], hif[:, :], 256.0, None, AL_.mult)
        nc.vector.tensor_tensor(hif[:, :], hif[:, :], lof[:, :], AL_.add)
        nc.vector.tensor_scalar(out3[:, :, i], hif[:, :], 512.0, None,
                                AL_.subtract)


def _build():
    if "nc" in _CACHE:
        return _CACHE["nc"]
    nc = bacc.Bacc(None, target_bir_lowering=False, debug=False)

    fb_pk = nc.declare_dram_parameter("fb", [9, 128, FLATC // 8], U8,
                                      isOutput=False)
    idx_in = nc.declare_dram_parameter("idx", [16, NGRP * 600], I16,
                                       isOutput=False)
    cw_pk = nc.declare_dram_parameter("cw", [5, 128, NGRP * 150 // 4], U8,
                                      isOutput=False)
    pgw_pk = nc.declare_dram_parameter("pgw", [2, 9, 128, 1024], U8,
                                       isOutput=False)
    pgb_in = nc.declare_dram_parameter("pgb", [1, 8192], BF16, isOutput=False)
    qft_pk = nc.declare_dram_parameter("qft", [2, 5, 128, QT // 4], U8,
                                       isOutput=False)
    opw_pk = nc.declare_dram_parameter("opw", [CG, 9, 128, D // 8], U8,
                                       isOutput=False)
    scl_in = nc.declare_dram_parameter("scl", [128, 2], F32, isOutput=False)
    e2_in = nc.declare_dram_parameter("e2", [64, 2], F32, isOutput=False)
    e2t_in = nc.declare_dram_parameter("e2t", [2, 64], F32, isOutput=False)
    out_ext = nc.declare_dram_parameter("out", [2, 128, QT], BF16,
                                        isOutput=True)

    with tile.TileContext(nc) as tc:
        with (
            tc.tile_pool(name="dram", bufs=1, space="DRAM") as dp,
            tc.tile_pool(name="const", bufs=1) as cp,
        ):
            fs32 = dp.tile([PIXPAD, CG], F32, tag="fs32")
            pdram = dp.tile([QT, 8192], BF16, tag="pdram")
            opw_dec = dp.tile([CG, 128, D], BF16, tag="opw_dec")

            ident = cp.tile([128, 128], F32, tag="ident")
            make_identity(nc, ident[:, :])
            e2 = cp.tile([64, 2], F32, tag="e2")
            nc.sync.dma_start(e2[:, :], e2_in[:, :])
            e2t = cp.tile([2, 64], F32, tag="e2t")
            nc.sync.dma_start(e2t[:, :], e2t_in[:, :])
            ones_f = cp.tile([1, 128], F32, tag="ones_f")
            nc.vector.memset(ones_f[:, :], 1.0)
            ones_b = cp.tile([1, 128], BF16, tag="ones_b")
            nc.vector.memset(ones_b[:, :], 1.0)
            onesc_f = cp.tile([128, 1], F32, tag="onesc_f")
            nc.vector.memset(onesc_f[:, :], 1.0)
            cw = cp.tile([128, NGRP * 150], BF16, tag="cw")
            scl = cp.tile([128, 2], F32, tag="scl")
            nc.sync.dma_start(scl[:, :], scl_in[:, :])
            # decode 10-bit cw planes, fold in 1/(s_cw*s_fb)
            with tc.tile_pool(name="pcw", bufs=1) as pcw:
                NQC = NGRP * 150 // 4
                plc = pcw.tile([128, 5, NQC], U8, tag="plc")
                for i in range(5):
                    nc.sync.dma_start(plc[:, i, :], cw_pk[i])
                cwf = pcw.tile([128, NQC, 4], F32, tag="cwf")
                _dec10(nc, pcw, plc, cwf, NQC, tagp="c")
                nc.vector.tensor_tensor(
                    cw[:, :].rearrange("p (a b) -> p a b", b=4),
                    cwf[:, :, :],
                    scl[:, 1:2].unsqueeze(2).to_broadcast([128, NQC, 4]),
                    AL.mult)

            # ---- Phase A: decode 9-bit features to f32 gather source ----
            fs_flat = fs32[:, :].rearrange("r c -> (r c)").rearrange(
                "(p i) -> p i", p=128)
            fs_flat3 = fs_flat.rearrange("p (a b) -> p a b", b=8)
            with tc.tile_pool(name="pa", bufs=2) as pa:
                NO = FLATC // 8          # 5320 octets per partition
                CH = NO // 8
                for ch in range(8):
                    sl = slice(ch * CH, (ch + 1) * CH)
                    pl = pa.tile([128, 9, CH], U8, tag="pl")
                    for i in range(9):
                        nc.sync.dma_start(pl[:, i, :], fb_pk[i][:, sl])
                    t32 = pa.tile([128, CH, 8], F32, tag="t32")
                    _dec9(nc, pa, pl, t32, CH, tagp="a")
                    nc.sync.dma_start(fs_flat3[:, sl, :], t32[:, :, :])

            # ---- opw decode prologue: u8 planes -> bf16 in DRAM ----
            with tc.tile_pool(name="pod", bufs=2) as pod:
                for d0 in range(0, CG, 4):
                    pl = pod.tile([128, 9, 4 * (D // 8)], U8, tag="plo")
                    for i in range(9):
                        nc.sync.dma_start(
                            pl[:, i, :].rearrange("p (d c) -> p d c", d=4),
                            opw_pk[d0:d0 + 4, i].rearrange("d p c -> p d c"))
                    owf = pod.tile([128, 4 * (D // 8), 8], BF16, tag="owf")
                    _dec9(nc, pod, pl, owf, 4 * (D // 8), tagp="o")
                    nc.sync.dma_start(
                        opw_dec[d0:d0 + 4].rearrange("d p c -> p d c"),
                        owf[:, :, :].rearrange("p (d c) f -> p d (c f)", d=4))

            # ---- Phase B: params GEMM -> pdram [QT, 8192] (q-major, bf16) --
            with (
                tc.tile_pool(name="pb", bufs=2) as pb,
                tc.tile_pool(name="pbw", bufs=1) as pbw,
                tc.tile_pool(name="psb", bufs=4, space="PSUM") as psb,
            ):
                pgw_sb = []
                for k in range(2):
                    w = pbw.tile([128, 8192], BF16, tag=f"pgw{k}")
                    w3 = w[:, :].rearrange("p (a b) -> p a b", b=8)
                    for ch in range(4):
                        sl = slice(ch * 256, (ch + 1) * 256)
                        pl = pb.tile([128, 9, 256], U8, tag="plw")
                        for i in range(9):
                            nc.sync.dma_start(pl[:, i, :], pgw_pk[k, i][:, sl])
                        _dec9(nc, pb, pl, w3[:, sl, :], 256, tagp="w")
                    pgw_sb.append(w)
                pgb_sb = pbw.tile([1, 8192], BF16, tag="pgb")
                nc.sync.dma_start(pgb_sb[:, :], pgb_in[:, :])
                qft_sb = []
                for k in range(2):
                    w = pbw.tile([128, QT], BF16, tag=f"qft{k}")
                    plq = pb.tile([128, 5, QT // 4], U8, tag="plq")
                    for i in range(5):
                        nc.sync.dma_start(plq[:, i, :], qft_pk[k, i])
                    _dec10(nc, pb,
                           plq, w[:, :].rearrange("p (a b) -> p a b", b=4),
                           QT // 4, tagp="q")
                    qft_sb.append(w)
                for qb in range(10):
                    qs = slice(qb * 120, (qb + 1) * 120)
                    qsb = pb.tile([120, 8192], BF16, tag="qsb")
                    for cb in range(16):
                        cs = slice(cb * 512, (cb + 1) * 512)
                        ps = psb.tile([120, 512], F32, tag="ps")
                        nc.tensor.matmul(ps[:, :], qft_sb[0][:, qs],
                                         pgw_sb[0][:, cs], start=True,
                                         stop=False)
                        nc.tensor.matmul(ps[:, :], qft_sb[1][:, qs],
                                         pgw_sb[1][:, cs], start=False,
                                         stop=False)
                        nc.tensor.matmul(ps[:, :], ones_b[0:1, :120],
                                         pgb_sb[0:1, cs], start=False,
                                         stop=True)
                        # evac with 1/s_pgw descale (scale is a per-core input)
                        nc.vector.tensor_tensor(
                            qsb[:, cs], ps[:, :],
                            scl[:120, 0:1].to_broadcast([120, 512]), AL.mult)
                    nc.sync.dma_start(pdram[qs, :], qsb[:, :])

            # ---- Phase C/D per image ----
            from contextlib import ExitStack
            with ExitStack() as stack:
                pool = lambda n, b, **kw: stack.enter_context(
                    tc.tile_pool(name=n, bufs=b, **kw))
                pidx = pool("pidx", 1)
                pg = pool("pg", 2)
                pacc = pool("pacc", 1)
                pms = pool("pms", 2)
                pstp = pool("pst", 3)
                ph = pool("ph", 1)
                psqp = pool("psq", 1)
                psm = pool("psmall", 2)
                pdw = pool("pdw", 3)
                pout = pool("pout", 2)
                psc = pool("psc", 1, space="PSUM")
                psh2 = pool("psh2", 2, space="PSUM")
                psc2 = pool("psc2", 1, space="PSUM")
                psms = pool("psms", 1, space="PSUM")
                pso = pool("pso", 1, space="PSUM")
                for img in range(B):
                    qoff = img * QI
                    idx_sb = pidx.tile([128, 8 * 600], I16, tag="idx")
                    for r in range(8):
                        nc.sync.dma_start(
                            idx_sb[r * 16:(r + 1) * 16, :],
                            idx_in[:, img * 4800:(img + 1) * 4800])

                    acc = pacc.tile([128, 75, CG], F32, tag="acc")
                    for grp in range(8):
                        li, dy = grp // 2, grp % 2
                        a = img * PIX_IMG + LVL_BASE[li]
                        rows = LVL_ROWS[li] + 2
                        base_ap = fs32[a:a + rows, :]
                        pair = AP(base_ap.tensor, base_ap.offset,
                                  [(CG, rows), (1, 2 * CG)])
                        ci = img * 8 + grp
                        for half in range(3):
                            c0 = 25 * half
                            ncol = 25
                            v = pg.tile([128, 25, 128], F32, tag="v")
                            off = 0
                            while off < 25:
                                nn = min(1024, (25 - off) * 128)
                                i0 = grp * 600 + (c0 + off) * 8
                                nc.gpsimd.dma_gather(
                                    v[:, off:off + nn // 128, :],
                                    pair,
                                    idx_sb[:, i0:i0 + nn // 16],
                                    nn, nn, 2 * CG, elem_step=CG)
                                off += nn // 128
                            v4 = v[:, 0:ncol, :].rearrange(
                                "p a (s c) -> p a s c", s=2)
                            wexp = cw[:, ci * 150 + c0 * 2:
                                      ci * 150 + (c0 + ncol) * 2].rearrange(
                                "p (a s) -> p a s", s=2).unsqueeze(
                                3).to_broadcast([128, ncol, 2, CG])
                            nc.vector.tensor_tensor(v4, v4, wexp, AL.mult)
                            sl = acc[:, c0:c0 + ncol, :]
                            if grp == 0:
                                nc.vector.tensor_tensor(
                                    sl, v[:, 0:ncol, 0:CG],
                                    v[:, 0:ncol, CG:], AL.add)
                            else:
                                nc.vector.tensor_tensor(
                                    sl, sl, v[:, 0:ncol, 0:CG], AL.add)
                                nc.vector.tensor_tensor(
                                    sl, sl, v[:, 0:ncol, CG:], AL.add)

                    # ---- mix1: per-query sampled @ M (queries 4-stacked on
                    # partitions by the gather layout) ----
                    h1A = ph.tile([CG, 75, CG], BF16, tag="h1A")
                    h1B = ph.tile([CG, 75, CG], BF16, tag="h1B")
                    for qcb in range(15):
                        mi = pms.tile([CG, 20, CG], BF16, tag="mi")
                        nc.sync.dma_start(
                            mi[:, :, :],
                            pdram[qoff + qcb * 20:qoff + (qcb + 1) * 20,
                                  0:4096].rearrange("i (c d) -> c i d", c=CG))
                        h1psA = psc.tile([CG, 5, CG], F32, tag="h1psA")
                        h1psB = psc.tile([CG, 5, CG], F32, tag="h1psB")
                        for j in range(5):
                            qc = qcb * 5 + j
                            pst = psc2.tile([CG, 128], F32, tag="pst")
                            nc.tensor.transpose(pst[:, :], acc[:, qc, :],
                                                ident[:, :])
                            sT = pstp.tile([CG, 128], BF16, tag="sT")
                            nc.any.tensor_copy(sT[:, :], pst[:, :])
                            for q4 in range(4):
                                hp = h1psA if q4 < 2 else h1psB
                                pb_ = (q4 % 2) * PIN
                                nc.tensor.matmul(
                                    hp[pb_:pb_ + PIN, j, :],
                                    sT[:, q4 * PIN:(q4 + 1) * PIN],
                                    mi[:, j * 4 + q4, :],
                                    start=True, stop=True)
                        nc.any.tensor_copy(h1A[:, qcb * 5:(qcb + 1) * 5, :],
                                           h1psA[:, :, :])
                        nc.any.tensor_copy(h1B[:, qcb * 5:(qcb + 1) * 5, :],
                                           h1psB[:, :, :])

                    # LN#1: mean-center per query (rsqrt folded out), relu
                    h1rs = []
                    for hi, h1h in enumerate((h1A, h1B)):
                        h1d = psm.tile([CG, 75], F32, tag="h1d")
                        nc.vector.tensor_reduce(h1d[:, :].unsqueeze(2),
                                                h1h[:, :, :], AX.X, AL.add)
                        s1p = psms.tile([128, QI], F32, tag="pmm")
                        nc.tensor.matmul(s1p[:2, :75], e2[:, :], h1d[:, :],
                                         start=True, stop=True)
                        mu1 = psm.tile([2, 75], F32, tag="mu1")
                        nc.any.tensor_scalar(mu1[:, :], s1p[:2, :75],
                                             1.0 / 2048.0, None, AL.mult)
                        m1e = psms.tile([128, QI], F32, tag="pmm")
                        nc.tensor.matmul(m1e[:CG, :75], e2t[:, :], mu1[:, :],
                                         start=True, stop=True)
                        mu1e = psm.tile([CG, 75], F32, tag="mu1e")
                        nc.any.tensor_copy(mu1e[:, :], m1e[:CG, :75])
                        for hq in range(2):
                            q4 = hi * 2 + hq
                            pb_ = hq * PIN
                            h1r = ph.tile([PIN, 75, CG], BF16,
                                          tag=f"h1rq{q4}")
                            nc.vector.tensor_tensor(
                                h1r[:, :, :], h1h[pb_:pb_ + PIN, :, :],
                                mu1e[pb_:pb_ + PIN, :].unsqueeze(
                                    2).to_broadcast([PIN, 75, CG]),
                                AL.subtract)
                            nc.any.tensor_scalar(
                                h1r[:, :, :].rearrange("p a b -> p (a b)"),
                                h1r[:, :, :].rearrange("p a b -> p (a b)"),
                                0.0, None, AL.max)
                            h1rs.append(h1r)

                    # ---- mix2: h2[q] = S_q @ h1r_q -> h2sb [128 o, 300, 64]
                    h2sb = ph.tile([128, QI, CG], BF16, tag="h2sb")
                    for qcb in range(15):
                        blk = pdram[qoff + qcb * 20:qoff + (qcb + 1) * 20,
                                    4096:8192].rearrange(
                            "(i q) (p o) -> q p i o", i=5, p=PIN)
                        sis = []
                        for q4 in range(4):
                            si = pms.tile([PIN, 5, 128], BF16,
                                          tag=f"siq{q4}")
                            nc.sync.dma_start(si[:, :, :], blk[q4])
                            sis.append(si)
                        for jj in range(4):
                            h2ps = psh2.tile([128, 5, CG], F32, tag="h2ps")
                            for j in range(5):
                                i20 = jj * 5 + j
                                i5 = i20 // 4
                                qc = qcb * 5 + i5
                                q4 = i20 % 4
                                nc.tensor.matmul(
                                    h2ps[:, j, :],
                                    sis[q4][:, i5, :],
                                    h1rs[q4][:, qc, :],
                                    start=True, stop=True)
                            nc.any.tensor_copy(
                                h2sb[:, qcb * 20 + jj * 5:
                                     qcb * 20 + (jj + 1) * 5, :],
                                h2ps[:, :, :])

                    # LN#2 stats (over o,d per query)
                    h2d = psm.tile([128, QI], F32, tag="h2d")
                    nc.vector.tensor_reduce(h2d[:, :].unsqueeze(2),
                                            h2sb[:, :, :], AX.X, AL.add)
                    sqd2 = psm.tile([128, QI], F32, tag="sqd2")
                    for kk in range(12):
                        sl = slice(kk * 25, (kk + 1) * 25)
                        sq2 = psqp.tile([128, 25 * CG], F32, tag="sq")
                        nc.scalar.activation(
                            sq2[:, :],
                            h2sb[:, sl, :].rearrange("p a b -> p (a b)"),
                            AF.Square)
                        nc.vector.tensor_reduce(
                            sqd2[:, sl].unsqueeze(2),
                            sq2[:, :].rearrange("p (a b) -> p a b", b=CG),
                            AX.X, AL.add)
                    s1q = psms.tile([128, QI], F32, tag="pmm")
                    nc.tensor.matmul(s1q[:1, :], onesc_f[:, :], h2d[:, :],
                                     start=True, stop=True)
                    s2q = psms.tile([128, QI], F32, tag="pmm")
                    nc.tensor.matmul(s2q[:1, :], onesc_f[:, :], sqd2[:, :],
                                     start=True, stop=True)
                    mu2 = psm.tile([1, QI], F32, tag="mu2")
                    nc.any.tensor_scalar(mu2[:, :], s1q[:1, :], 1.0 / 8192.0,
                                         None, AL.mult)
                    ex2 = psm.tile([1, QI], F32, tag="ex2")
                    nc.any.tensor_scalar(ex2[:, :], s2q[:1, :], 1.0 / 8192.0,
                                         None, AL.mult)
                    var2 = psm.tile([1, QI], F32, tag="var2")
                    nc.vector.tensor_tensor(var2[:, :], mu2[:, :], mu2[:, :],
                                            AL.mult)
                    nc.vector.tensor_tensor(var2[:, :], ex2[:, :], var2[:, :],
                                            AL.subtract)
                    r2 = psm.tile([1, QI], F32, tag="r2")
                    nc.any.tensor_scalar(var2[:, :], var2[:, :], 1e-5,
                                         None, AL.add)
                    nc.scalar.activation(r2[:, :], var2[:, :], AF.Sqrt)
                    nc.vector.reciprocal(r2[:, :], r2[:, :])
                    m2e = psms.tile([128, QI], F32, tag="pmm")
                    nc.tensor.matmul(m2e[:, :], ones_f[:, :], mu2[:, :],
                                     start=True, stop=True)
                    mu2e = psm.tile([128, QI], F32, tag="mu2e")
                    nc.any.tensor_copy(mu2e[:, :], m2e[:, :])
                    r2ep = psms.tile([128, QI], F32, tag="pmm")
                    nc.tensor.matmul(r2ep[:, :], ones_f[:, :], r2[:, :],
                                     start=True, stop=True)
                    r2e = psm.tile([128, QI], F32, tag="r2e")
                    nc.any.tensor_copy(r2e[:, :], r2ep[:, :])

                    # h2r = relu(h2 - mu2) in place
                    nc.vector.tensor_tensor(
                        h2sb[:, :, :], h2sb[:, :, :],
                        mu2e[:, :].unsqueeze(2).to_broadcast([128, QI, CG]),
                        AL.subtract)
                    nc.any.tensor_scalar(
                        h2sb[:, :, :].rearrange("p a b -> p (a b)"),
                        h2sb[:, :, :].rearrange("p a b -> p (a b)"),
                        0.0, None, AL.max)

                    # ---- Phase D: projection, contract over o with PSUM
                    # accumulation over d; h2sb stays in SBUF ----
                    pr0 = pso.tile([128, QI], F32, tag="pr0")
                    pr1 = pso.tile([128, QI], F32, tag="pr1")
                    prps = [pr0, pr1]
                    for d in range(CG):
                        ow = pdw.tile([128, D], BF16, tag="ow")
                        nc.sync.dma_start(ow[:, :], opw_dec[d])
                        for dh in range(2):
                            nc.tensor.matmul(
                                prps[dh][:, :],
                                ow[:, dh * 128:(dh + 1) * 128],
                                h2sb[:, :, d],
                                start=(d == 0), stop=(d == CG - 1))
                    for dh in range(2):
                        osb = pout.tile([128, QI], BF16, tag="osb")
                        nc.vector.tensor_tensor(
                            osb[:, :], prps[dh][:, :], r2e[:, :], AL.mult)
                        nc.sync.dma_start(
                            out_ext[dh, :, qoff:qoff + QI], osb[:, :])
    nc.compile()
    _CACHE["nc"] = nc
    return nc


def _host_prep(feats, query_feat, query_roi, off_w, off_b, pg_w, pg_b, op_w):
    """Vectorized numpy: addressing metadata + per-core input tensors."""
    qf = query_feat.astype(np.float32)
    offset = (qf @ off_w + off_b).reshape(B, N, G * PIN, 3)
    roi_cc = query_roi[..., :2]
    scale = 2.0 ** query_roi[..., 2:3]
    ratio = 2.0 ** np.concatenate(
        [query_roi[..., 3:4] * -0.5, query_roi[..., 3:4] * 0.5], axis=-1)
    roi_wh = scale * ratio
    sample_xy = roi_cc[:, :, None, :] + offset[..., :2] * roi_wh[:, :, None, :]
    sample_z = query_roi[..., 2:3] + offset[..., 2]
    lvl = np.arange(4, dtype=np.float32)
    logits = -((sample_z - MAP_STRIDE)[..., None] - lvl) ** 2 / TAU
    logits -= logits.max(-1, keepdims=True)
    e = np.exp(logits)
    lw = (e / e.sum(-1, keepdims=True)).astype(np.float32)  # [B,N,G*PIN,4]
    sx = sample_xy[..., 0]                                  # [B,N,G*PIN]
    sy = sample_xy[..., 1]

    # per (lvl, ycorner): pair base index + 2 slot weights, [B, N, G*PIN]
    idx_all = np.zeros((4, 2, B, N, G * PIN), np.int16)
    w_all = np.zeros((4, 2, 2, B, N, G * PIN), np.float32)
    for li, ((H, W), stride) in enumerate(zip(SIZES, STRIDES)):
        px = sx / stride - 0.5
        py = sy / stride - 0.5
        x0 = np.floor(px)
        y0 = np.floor(py)
        wx1 = (px - x0).astype(np.float32)
        wy1 = (py - y0).astype(np.float32)
        vx0 = (x0 >= 0) & (x0 <= W - 1)
        vx1 = (x0 + 1 >= 0) & (x0 + 1 <= W - 1)
        xc = np.clip(x0, -1, W - 1)
        for dy in range(2):
            yd = y0 + dy
            vy = (yd >= 0) & (yd <= H - 1)
            yc = np.clip(yd, 0, H - 1)
            wy = (wy1 if dy else 1.0 - wy1) * lw[..., li]
            idx_all[li, dy] = (yc * W + xc + 1).astype(np.int16)
            w_all[li, dy, 0] = (1.0 - wx1) * wy * (vx0 & vy)
            w_all[li, dy, 1] = wx1 * wy * (vx1 & vy)

    ew = np.zeros((64, 2), np.float32)
    for p in range(64):
        ew[p, p // 32] = 1.0

    # S column order: device col 4096 + p*128 + o holds S[o, p]
    p_idx = np.repeat(np.arange(PIN), 128)
    o_idx = np.tile(np.arange(128), PIN)
    scol = 4096 + o_idx * PIN + p_idx
    cols = np.concatenate([np.arange(4096), scol])

    qft_pk, s_qft = _pack10(
        np.ascontiguousarray(qf.reshape(QT, D).T.reshape(2, 128, QT)),
        plane_axis=1)

    in_maps = []
    for g in range(G):
        # features: flat [pad, img0 lvls, img1..., pad] x 64ch, 10-bit packed
        fparts = [np.zeros((1, CG), np.float32)]
        for b in range(B):
            for li, (H, W) in enumerate(SIZES):
                f = feats[li][b, g * CG:(g + 1) * CG]      # [64, H, W]
                fparts.append(f.reshape(CG, H * W).T)
        fcat = np.concatenate(fparts, axis=0)
        pad_tail = PIXPAD - fcat.shape[0]
        fcat = np.concatenate(
            [fcat, np.zeros((pad_tail, CG), np.float32)], axis=0)
        fb_pk, s_fb = _pack9(fcat.reshape(128, FLATC), plane_axis=0)

        idx_cols = np.empty((16, NGRP * 600), np.int16)
        cw_cols = np.empty((128, NGRP * 150), np.float32)
        for b in range(B):
            for li in range(4):
                for dy in range(2):
                    ci = b * 8 + li * 2 + dy
                    ia = idx_all[li, dy, b, :, g * PIN:(g + 1) * PIN]
                    idx_cols[:, ci * 600:(ci + 1) * 600] = \
                        ia.reshape(NSMP).reshape(600, 16).T
                    wp = np.stack(
                        [w_all[li, dy, s, b, :, g * PIN:(g + 1) * PIN]
                         .reshape(NSMP) for s in range(2)], axis=-1)
                    cw_cols[:, ci * 150:(ci + 1) * 150] = \
                        wp.reshape(75, 128, 2).transpose(1, 0, 2).reshape(
                            128, 150)
        cw_pk, s_cw = _pack10(cw_cols, plane_axis=0)

        pgw_c = pg_w[:, g * 8192:(g + 1) * 8192][:, cols].astype(
            np.float32).reshape(2, 128, 8192)
        pgw_pk, s_pgw = _pack9(pgw_c, plane_axis=1)
        pgb_c = (pg_b[g * 8192:(g + 1) * 8192][cols] * (s_pgw * s_qft)
                 ).astype(BF)[None, :]
        scl_c = np.stack([
            np.full(128, 1.0 / (s_pgw * s_qft), np.float32),
            np.full(128, 1.0 / (s_cw * s_fb), np.float32)], axis=1)
        opw_c = op_w[g * 8192:(g + 1) * 8192, :].reshape(128, CG, D)\
            .transpose(1, 0, 2).astype(np.float32)
        opw_pk, s_opw = _pack9(opw_c, plane_axis=1)
        in_maps.append({
            "fb": fb_pk, "idx": np.ascontiguousarray(idx_cols),
            "cw": cw_pk,
            "pgw": pgw_pk, "pgb": np.ascontiguousarray(pgb_c),
            "qft": qft_pk, "opw": opw_pk, "scl": scl_c,
            "e2": ew, "e2t": np.ascontiguousarray(ew.T),
            "_host_s_opw": s_opw,      # host-only: not a declared input
        })
    return in_maps


def kernel(feat0, feat1, feat2, feat3, query_feat, query_roi,
           off_w, off_b, pg_w, pg_b, op_w, op_b, ln_g, ln_b):
    feats = [np.asarray(f, np.float32) for f in (feat0, feat1, feat2, feat3)]
    query_feat = np.asarray(query_feat, np.float32)
    query_roi = np.asarray(query_roi, np.float32)
    in_maps = _host_prep(feats, query_feat, query_roi,
                         np.asarray(off_w, np.float32),
                         np.asarray(off_b, np.float32),
                         np.asarray(pg_w, np.float32),
                         np.asarray(pg_b, np.float32),
                         np.asarray(op_w, np.float32))
    nc = _build()
    cores = list(range(G))
    if not _CACHE.get("warm"):
        # first call compiles/loads the PJRT executable; run it once so the
        # steady-state call below reflects transfer+execute only
        run_bass_kernel_spmd(nc, in_maps, core_ids=cores)
        _CACHE["warm"] = True
    res = run_bass_kernel_spmd(nc, in_maps, core_ids=cores)
    outs = res.results

    op_b = np.asarray(op_b, np.float32)
    ln_g = np.asarray(ln_g, np.float32)
    ln_b = np.asarray(ln_b, np.float32)
    acc = np.zeros((D, QT), np.float32)
    for g in range(G):
        o = outs[g]
        o = o["out"] if isinstance(o, dict) else o[0]
        acc += np.asarray(o, np.float32).reshape(D, QT) / \
            in_maps[g]["_host_s_opw"]
    h = acc.T.reshape(B, N, D) + query_feat + op_b
    mu = h.mean(-1, keepdims=True)
    var = ((h - mu) ** 2).mean(-1, keepdims=True)
    return (h - mu) / np.sqrt(var + 1e-5) * ln_g + ln_b


# revision 41
# speedup vs baseline: 1.1767x; 1.1767x over previous
"""AdaptiveSamplingMixing — Trainium2 SPMD kernel (4 cores, group-parallel).

Measurement reality on this setup: the axon tunnel moves host<->device data at
~50 MB/s while the device kernel itself executes in ~2 ms/core, so the
end-to-end run_bass_kernel_spmd call is dominated by input upload.  The kernel
is organised to minimise uploaded bytes (123 MB baseline -> ~58 MB):

  - core = g (one sampling group per core, all 4 images).  The previous
    (image-pair x group) split duplicated pg_w/op_w across image pairs
    (+33 MB); group-sharding uploads every weight byte exactly once.
  - features, pg_w and op_w upload as absmax-scaled 9-bit fixed point
    (8 values in 9 u8 planes); cw and qft as 10-bit (4 values in 5 planes).
    The vector engine decodes (shift/and/scale) on device.  The feature
    descale folds into the bilinear weights, pg_w's into the params-GEMM
    psum evacuation, op_w's into the host-side group sum, so dequantisation
    costs no extra device passes.  End-to-end error ~1.1e-2 (vs ~5.2e-3 for
    all-bf16) against the 2e-2 gate; the numpy emulation of this pipeline
    predicts hardware error to ~1e-4, so the bit widths were chosen from a
    measured error/bytes sweep (fp8 fails at 2-5e-2: coherent-sum effect).
    Touched-pixel feature compaction was evaluated and shelved: the touched
    fraction swings 27%..75% across same-distribution PRNG draws, so a safe
    capacity erases the saving.
  - bilinear x-corner PAIRS are fetched with a single 512 B dma_gather
    descriptor via an overlapping strided view of the f32 feature buffer
    (elem_step=64 elems = 256 B, elem_size=128), halving gather descriptors
    and halving the uploaded index tables.
  - the output projection contracts over the POUT partition axis with PSUM
    accumulation over d, keeping h2 in SBUF (no DRAM round-trip / transposing
    regather).
  - partial outputs download as bf16.
  - kernel() runs the spmd call twice (first call warms the jax/PJRT
    executable cache); the steady-state call is what a benchmark observes.

Numerics: rsqrt factors of both inner layernorms are folded out algebraically
(LN2(r*X) == LN2(X) for per-query scales); mix2's r2 is applied as a per-query
column scale after the projection matmul.  The host computes addressing
metadata (sample indices / bilinear weights), packs/reshapes inputs, and
finishes with the 4-way group partial-sum + residual + final affine LayerNorm.
"""
import sys
sys.path.insert(0, "/opt/trn_rl_repo")
import numpy as np
import ml_dtypes

import concourse.bass as bass
import concourse.mybir as mybir
import concourse.tile as tile
from concourse import bacc
from concourse.ap import AP
from concourse.bass_utils import run_bass_kernel_spmd
from concourse.masks import make_identity

F32 = mybir.dt.float32
BF16 = mybir.dt.bfloat16
I16 = mybir.dt.int16
U8 = mybir.dt.uint8
AL = mybir.AluOpType
AF = mybir.ActivationFunctionType
AX = mybir.AxisListType
BF = ml_dtypes.bfloat16

B, N, D = 4, 300, 256
G, PIN, POUT = 4, 32, 128
CG = D // G  # 64
STRIDES = (8, 16, 32, 64)
SIZES = ((100, 160), (50, 80), (25, 40), (13, 20))
TAU, MAP_STRIDE = 2.0, 3.0

QI = 300                   # queries per image
QT = B * QI                # 1200 queries per core (all images)
PIX_IMG = sum(h * w for h, w in SIZES)           # 21260 pixels per image
LVL_BASE = (0, 16000, 20000, 21000)
LVL_ROWS = (16000, 4000, 1000, 260)
PIXPAD = 128 * 665         # 85120 >= 1 + 4*21260 + 1 (lead/tail pads)
FLATC = PIXPAD * CG // 128                       # 42560 flat cols of fb
NSMP = QI * PIN            # 9600 gather indices per (img, lvl, ycorner)
NGRP = 32                  # gather groups per core: img(4) x lvl(4) x y(2)

_CACHE = {}


def _pack10(x, plane_axis):
    """absmax-scaled signed 10-bit fixed point; 4 elems -> 5 u8 planes.

    Packs along the last axis (length divisible by 4).  Returns (planes u8
    with a new length-5 axis at plane_axis, scale s) where
    dequant = (int10 + 512 offset removed) / s.
    """
    x = np.asarray(x, np.float32)
    amax = float(np.abs(x).max())
    if not np.isfinite(amax) or amax == 0.0:
        amax = 1.0
    s = 511.0 / amax
    q = (np.clip(np.round(x * s), -512, 511).astype(np.int16) + 512).astype(
        np.uint16)
    q = q.reshape(*x.shape[:-1], x.shape[-1] // 4, 4)
    L = (q & 255).astype(np.uint8)
    H = ((q[..., 0] >> 8) | ((q[..., 1] >> 8) << 2) |
         ((q[..., 2] >> 8) << 4) | ((q[..., 3] >> 8) << 6)).astype(np.uint8)
    planes = np.stack([L[..., 0], L[..., 1], L[..., 2], L[..., 3], H],
                      axis=plane_axis)
    return np.ascontiguousarray(planes), s


def _pack9(x, plane_axis):
    """absmax-scaled signed 9-bit fixed point; 8 elems -> 9 u8 planes."""
    x = np.asarray(x, np.float32)
    amax = float(np.abs(x).max())
    if not np.isfinite(amax) or amax == 0.0:
        amax = 1.0
    s = 255.0 / amax
    q = (np.clip(np.round(x * s), -256, 255).astype(np.int16) + 256).astype(
        np.uint16)
    q = q.reshape(*x.shape[:-1], x.shape[-1] // 8, 8)
    L = (q & 255).astype(np.uint8)
    H = np.zeros(q.shape[:-1], np.uint8)
    for i in range(8):
        H |= ((q[..., i] >> 8) << i).astype(np.uint8)
    planes = np.stack([L[..., i] for i in range(8)] + [H], axis=plane_axis)
    return np.ascontiguousarray(planes), s


def _dec9(nc, pool, planes, out3, c, tagp=""):
    """device decode: planes [128,9,c] u8 -> out3 [128,c,8] float view."""
    for i in range(8):
        hi8 = pool.tile([128, c], U8, tag=f"hi8{tagp}")
        if i == 0:
            nc.vector.tensor_scalar(hi8[:, :], planes[:, 8, :], 1, None,
                                    AL.bitwise_and)
        else:
            nc.vector.tensor_scalar(hi8[:, :], planes[:, 8, :], i, None,
                                    AL.logical_shift_right)
            nc.vector.tensor_scalar(hi8[:, :], hi8[:, :], 1, None,
                                    AL.bitwise_and)
        hif = pool.tile([128, c], F32, tag=f"hif{tagp}")
        nc.vector.tensor_copy(hif[:, :], hi8[:, :])
        lof = pool.tile([128, c], F32, tag=f"lof{tagp}")
        nc.vector.tensor_copy(lof[:, :], planes[:, i, :])
        nc.vector.tensor_scalar(hif[:, :], hif[:, :], 256.0, None, AL.mult)
        nc.vector.tensor_tensor(hif[:, :], hif[:, :], lof[:, :], AL.add)
        nc.vector.tensor_scalar(out3[:, :, i], hif[:, :], 256.0, None,
                                AL.subtract)


def _dec10(nc, pool, planes, out3, c, tagp=""):
    """device decode: planes [128,5,c] u8 -> out3 [128,c,4] (f32/bf16 view),
    values = int10 (offset removed), i.e. true*scale."""
    AL_ = AL
    for i in range(4):
        hi8 = pool.tile([128, c], U8, tag=f"hi8{tagp}")
        if i == 0:
            nc.vector.tensor_scalar(hi8[:, :], planes[:, 4, :], 3, None,
                                    AL_.bitwise_and)
        else:
            nc.vector.tensor_scalar(hi8[:, :], planes[:, 4, :], 2 * i,
                                    None, AL_.logical_shift_right)
            nc.vector.tensor_scalar(hi8[:, :], hi8[:, :], 3, None,
                                    AL_.bitwise_and)
        hif = pool.tile([128, c], F32, tag=f"hif{tagp}")
        nc.vector.tensor_copy(hif[:, :], hi8[:, :])
        lof = pool.tile([128, c], F32, tag=f"lof{tagp}")
        nc.vector.tensor_copy(lof[:, :], planes[:, i, :])
        nc.vector.tensor_scalar(hif[:, :], hif[:, :], 256.0, None, AL_.mult)
        nc.vector.tensor_tensor(hif[:, :], hif[:, :], lof[:, :], AL_.add)
        nc.vector.tensor_scalar(out3[:, :, i], hif[:, :], 512.0, None,
                                AL_.subtract)


def _build():
    if "nc" in _CACHE:
        return _CACHE["nc"]
    nc = bacc.Bacc(None, target_bir_lowering=False, debug=False)

    fb_pk = nc.declare_dram_parameter("fb", [9, 128, FLATC // 8], U8,
                                      isOutput=False)
    idx_in = nc.declare_dram_parameter("idx", [16, NGRP * 600], I16,
                                       isOutput=False)
    cw_pk = nc.declare_dram_parameter("cw", [5, 128, NGRP * 150 // 4], U8,
                                      isOutput=False)
    pgw_pk = nc.declare_dram_parameter("pgw", [2, 9, 128, 1024], U8,
                                       isOutput=False)
    pgb_in = nc.declare_dram_parameter("pgb", [1, 8192], BF16, isOutput=False)
    qft_pk = nc.declare_dram_parameter("qft", [2, 5, 128, QT // 4], U8,
                                       isOutput=False)
    opw_pk = nc.declare_dram_parameter("opw", [CG, 9, 128, D // 8], U8,
                                       isOutput=False)
    scl_in = nc.declare_dram_parameter("scl", [128, 2], F32, isOutput=False)
    e2_in = nc.declare_dram_parameter("e2", [64, 2], F32, isOutput=False)
    e2t_in = nc.declare_dram_parameter("e2t", [2, 64], F32, isOutput=False)
    out_ext = nc.declare_dram_parameter("out", [2, 128, QT], BF16,
                                        isOutput=True)

    with tile.TileContext(nc) as tc:
        with (
            tc.tile_pool(name="dram", bufs=1, space="DRAM") as dp,
            tc.tile_pool(name="const", bufs=1) as cp,
        ):
            fs32 = dp.tile([PIXPAD, CG], F32, tag="fs32")
            pdram = dp.tile([QT, 8192], BF16, tag="pdram")
            opw_dec = dp.tile([CG, 128, D], BF16, tag="opw_dec")

            ident = cp.tile([128, 128], F32, tag="ident")
            make_identity(nc, ident[:, :])
            e2 = cp.tile([64, 2], F32, tag="e2")
            nc.sync.dma_start(e2[:, :], e2_in[:, :])
            e2t = cp.tile([2, 64], F32, tag="e2t")
            nc.sync.dma_start(e2t[:, :], e2t_in[:, :])
            ones_f = cp.tile([1, 128], F32, tag="ones_f")
            nc.vector.memset(ones_f[:, :], 1.0)
            ones_b = cp.tile([1, 128], BF16, tag="ones_b")
            nc.vector.memset(ones_b[:, :], 1.0)
            onesc_f = cp.tile([128, 1], F32, tag="onesc_f")
            nc.vector.memset(onesc_f[:, :], 1.0)
            cw = cp.tile([128, NGRP * 150], BF16, tag="cw")
            scl = cp.tile([128, 2], F32, tag="scl")
            nc.sync.dma_start(scl[:, :], scl_in[:, :])
            # decode 10-bit cw planes, fold in 1/(s_cw*s_fb)
            with tc.tile_pool(name="pcw", bufs=1) as pcw:
                NQC = NGRP * 150 // 4
                plc = pcw.tile([128, 5, NQC], U8, tag="plc")
                for i in range(5):
                    nc.sync.dma_start(plc[:, i, :], cw_pk[i])
                cwf = pcw.tile([128, NQC, 4], F32, tag="cwf")
                _dec10(nc, pcw, plc, cwf, NQC, tagp="c")
                nc.vector.tensor_tensor(
                    cw[:, :].rearrange("p (a b) -> p a b", b=4),
                    cwf[:, :, :],
                    scl[:, 1:2].unsqueeze(2).to_broadcast([128, NQC, 4]),
                    AL.mult)

            # ---- Phase A: decode 9-bit features to f32 gather source ----
            fs_flat = fs32[:, :].rearrange("r c -> (r c)").rearrange(
                "(p i) -> p i", p=128)
            fs_flat3 = fs_flat.rearrange("p (a b) -> p a b", b=8)
            with tc.tile_pool(name="pa", bufs=2) as pa:
                NO = FLATC // 8          # 5320 octets per partition
                CH = NO // 8
                for ch in range(8):
                    sl = slice(ch * CH, (ch + 1) * CH)
                    pl = pa.tile([128, 9, CH], U8, tag="pl")
                    for i in range(9):
                        nc.sync.dma_start(pl[:, i, :], fb_pk[i][:, sl])
                    t32 = pa.tile([128, CH, 8], F32, tag="t32")
                    _dec9(nc, pa, pl, t32, CH, tagp="a")
                    nc.sync.dma_start(fs_flat3[:, sl, :], t32[:, :, :])

            # ---- opw decode prologue: u8 planes -> bf16 in DRAM ----
            with tc.tile_pool(name="pod", bufs=2) as pod:
                for d0 in range(0, CG, 4):
                    pl = pod.tile([128, 9, 4 * (D // 8)], U8, tag="plo")
                    for i in range(9):
                        nc.sync.dma_start(
                            pl[:, i, :].rearrange("p (d c) -> p d c", d=4),
                            opw_pk[d0:d0 + 4, i].rearrange("d p c -> p d c"))
                    owf = pod.tile([128, 4 * (D // 8), 8], BF16, tag="owf")
                    _dec9(nc, pod, pl, owf, 4 * (D // 8), tagp="o")
                    nc.sync.dma_start(
                        opw_dec[d0:d0 + 4].rearrange("d p c -> p d c"),
                        owf[:, :, :].rearrange("p (d c) f -> p d (c f)", d=4))

            # ---- Phase B: params GEMM -> pdram [QT, 8192] (q-major, bf16) --
            with (
                tc.tile_pool(name="pb", bufs=2) as pb,
                tc.tile_pool(name="pbw", bufs=1) as pbw,
                tc.tile_pool(name="psb", bufs=4, space="PSUM") as psb,
            ):
                pgw_sb = []
                for k in range(2):
                    w = pbw.tile([128, 8192], BF16, tag=f"pgw{k}")
                    w3 = w[:, :].rearrange("p (a b) -> p a b", b=8)
                    for ch in range(4):
                        sl = slice(ch * 256, (ch + 1) * 256)
                        pl = pb.tile([128, 9, 256], U8, tag="plw")
                        for i in range(9):
                            nc.sync.dma_start(pl[:, i, :], pgw_pk[k, i][:, sl])
                        _dec9(nc, pb, pl, w3[:, sl, :], 256, tagp="w")
                    pgw_sb.append(w)
                pgb_sb = pbw.tile([1, 8192], BF16, tag="pgb")
                nc.sync.dma_start(pgb_sb[:, :], pgb_in[:, :])
                qft_sb = []
                for k in range(2):
                    w = pbw.tile([128, QT], BF16, tag=f"qft{k}")
                    plq = pb.tile([128, 5, QT // 4], U8, tag="plq")
                    for i in range(5):
                        nc.sync.dma_start(plq[:, i, :], qft_pk[k, i])
                    _dec10(nc, pb,
                           plq, w[:, :].rearrange("p (a b) -> p a b", b=4),
                           QT // 4, tagp="q")
                    qft_sb.append(w)
                for qb in range(10):
                    qs = slice(qb * 120, (qb + 1) * 120)
                    qsb = pb.tile([120, 8192], BF16, tag="qsb")
                    for cb in range(16):
                        cs = slice(cb * 512, (cb + 1) * 512)
                        ps = psb.tile([120, 512], F32, tag="ps")
                        nc.tensor.matmul(ps[:, :], qft_sb[0][:, qs],
                                         pgw_sb[0][:, cs], start=True,
                                         stop=False)
                        nc.tensor.matmul(ps[:, :], qft_sb[1][:, qs],
                                         pgw_sb[1][:, cs], start=False,
                                         stop=False)
                        nc.tensor.matmul(ps[:, :], ones_b[0:1, :120],
                                         pgb_sb[0:1, cs], start=False,
                                         stop=True)
                        # evac with 1/s_pgw descale (scale is a per-core input)
                        nc.vector.tensor_tensor(
                            qsb[:, cs], ps[:, :],
                            scl[:120, 0:1].to_broadcast([120, 512]), AL.mult)
                    nc.sync.dma_start(pdram[qs, :], qsb[:, :])

            # ---- Phase C/D per image ----
            from contextlib import ExitStack
            with ExitStack() as stack:
                pool = lambda n, b, **kw: stack.enter_context(
                    tc.tile_pool(name=n, bufs=b, **kw))
                pidx = pool("pidx", 1)
                pg = pool("pg", 1)
                pacc = pool("pacc", 1)
                pms = pool("pms", 2)
                pstp = pool("pst", 3)
                ph = pool("ph", 1)
                psqp = pool("psq", 1)
                psm = pool("psmall", 2)
                pdw = pool("pdw", 3)
                pout = pool("pout", 2)
                psc = pool("psc", 1, space="PSUM")
                psh2 = pool("psh2", 2, space="PSUM")
                psc2 = pool("psc2", 1, space="PSUM")
                psms = pool("psms", 1, space="PSUM")
                pso = pool("pso", 1, space="PSUM")
                for img in range(B):
                    qoff = img * QI
                    idx_sb = pidx.tile([128, 8 * 600], I16, tag="idx")
                    for r in range(8):
                        nc.sync.dma_start(
                            idx_sb[r * 16:(r + 1) * 16, :],
                            idx_in[:, img * 4800:(img + 1) * 4800])

                    acc = pacc.tile([128, 75, CG], F32, tag="acc")
                    for grp in range(8):
                        li, dy = grp // 2, grp % 2
                        a = img * PIX_IMG + LVL_BASE[li]
                        rows = LVL_ROWS[li] + 2
                        base_ap = fs32[a:a + rows, :]
                        pair = AP(base_ap.tensor, base_ap.offset,
                                  [(CG, rows), (1, 2 * CG)])
                        ci = img * 8 + grp
                        for half in range(2):
                            c0 = 40 * half
                            ncol = 40 if half == 0 else 35
                            v = pg.tile([128, 40, 128], F32, tag="v")
                            for cc in range(5):
                                gc = 5 * half + cc
                                nn = 1024 if gc < 9 else 384
                                nc.gpsimd.dma_gather(
                                    v[:, cc * 8:cc * 8 + nn // 128, :],
                                    pair,
                                    idx_sb[:, grp * 600 + gc * 64:
                                           grp * 600 + gc * 64 + nn // 16],
                                    nn, nn, 2 * CG, elem_step=CG)
                            v4 = v[:, 0:ncol, :].rearrange(
                                "p a (s c) -> p a s c", s=2)
                            wexp = cw[:, ci * 150 + c0 * 2:
                                      ci * 150 + (c0 + ncol) * 2].rearrange(
                                "p (a s) -> p a s", s=2).unsqueeze(
                                3).to_broadcast([128, ncol, 2, CG])
                            nc.vector.tensor_tensor(v4, v4, wexp, AL.mult)
                            sl = acc[:, c0:c0 + ncol, :]
                            if grp == 0:
                                nc.vector.tensor_tensor(
                                    sl, v[:, 0:ncol, 0:CG],
                                    v[:, 0:ncol, CG:], AL.add)
                            else:
                                nc.vector.tensor_tensor(
                                    sl, sl, v[:, 0:ncol, 0:CG], AL.add)
                                nc.vector.tensor_tensor(
                                    sl, sl, v[:, 0:ncol, CG:], AL.add)

                    # ---- mix1: per-query sampled @ M (queries 4-stacked on
                    # partitions by the gather layout) ----
                    h1A = ph.tile([CG, 75, CG], BF16, tag="h1A")
                    h1B = ph.tile([CG, 75, CG], BF16, tag="h1B")
                    for qcb in range(15):
                        mi = pms.tile([CG, 20, CG], BF16, tag="mi")
                        nc.sync.dma_start(
                            mi[:, :, :],
                            pdram[qoff + qcb * 20:qoff + (qcb + 1) * 20,
                                  0:4096].rearrange("i (c d) -> c i d", c=CG))
                        h1psA = psc.tile([CG, 5, CG], F32, tag="h1psA")
                        h1psB = psc.tile([CG, 5, CG], F32, tag="h1psB")
                        for j in range(5):
                            qc = qcb * 5 + j
                            pst = psc2.tile([CG, 128], F32, tag="pst")
                            nc.tensor.transpose(pst[:, :], acc[:, qc, :],
                                                ident[:, :])
                            sT = pstp.tile([CG, 128], BF16, tag="sT")
                            nc.any.tensor_copy(sT[:, :], pst[:, :])
                            for q4 in range(4):
                                hp = h1psA if q4 < 2 else h1psB
                                pb_ = (q4 % 2) * PIN
                                nc.tensor.matmul(
                                    hp[pb_:pb_ + PIN, j, :],
                                    sT[:, q4 * PIN:(q4 + 1) * PIN],
                                    mi[:, j * 4 + q4, :],
                                    start=True, stop=True)
                        nc.any.tensor_copy(h1A[:, qcb * 5:(qcb + 1) * 5, :],
                                           h1psA[:, :, :])
                        nc.any.tensor_copy(h1B[:, qcb * 5:(qcb + 1) * 5, :],
                                           h1psB[:, :, :])

                    # LN#1: mean-center per query (rsqrt folded out), relu
                    h1rs = []
                    for hi, h1h in enumerate((h1A, h1B)):
                        h1d = psm.tile([CG, 75], F32, tag="h1d")
                        nc.vector.tensor_reduce(h1d[:, :].unsqueeze(2),
                                                h1h[:, :, :], AX.X, AL.add)
                        s1p = psms.tile([128, QI], F32, tag="pmm")
                        nc.tensor.matmul(s1p[:2, :75], e2[:, :], h1d[:, :],
                                         start=True, stop=True)
                        mu1 = psm.tile([2, 75], F32, tag="mu1")
                        nc.any.tensor_scalar(mu1[:, :], s1p[:2, :75],
                                             1.0 / 2048.0, None, AL.mult)
                        m1e = psms.tile([128, QI], F32, tag="pmm")
                        nc.tensor.matmul(m1e[:CG, :75], e2t[:, :], mu1[:, :],
                                         start=True, stop=True)
                        mu1e = psm.tile([CG, 75], F32, tag="mu1e")
                        nc.any.tensor_copy(mu1e[:, :], m1e[:CG, :75])
                        for hq in range(2):
                            q4 = hi * 2 + hq
                            pb_ = hq * PIN
                            h1r = ph.tile([PIN, 75, CG], BF16,
                                          tag=f"h1rq{q4}")
                            nc.vector.tensor_tensor(
                                h1r[:, :, :], h1h[pb_:pb_ + PIN, :, :],
                                mu1e[pb_:pb_ + PIN, :].unsqueeze(
                                    2).to_broadcast([PIN, 75, CG]),
                                AL.subtract)
                            nc.any.tensor_scalar(
                                h1r[:, :, :].rearrange("p a b -> p (a b)"),
                                h1r[:, :, :].rearrange("p a b -> p (a b)"),
                                0.0, None, AL.max)
                            h1rs.append(h1r)

                    # ---- mix2: h2[q] = S_q @ h1r_q -> h2sb [128 o, 300, 64]
                    h2sb = ph.tile([128, QI, CG], BF16, tag="h2sb")
                    for qcb in range(15):
                        blk = pdram[qoff + qcb * 20:qoff + (qcb + 1) * 20,
                                    4096:8192].rearrange(
                            "(i q) (p o) -> q p i o", i=5, p=PIN)
                        sis = []
                        for q4 in range(4):
                            si = pms.tile([PIN, 5, 128], BF16,
                                          tag=f"siq{q4}")
                            nc.sync.dma_start(si[:, :, :], blk[q4])
                            sis.append(si)
                        for jj in range(4):
                            h2ps = psh2.tile([128, 5, CG], F32, tag="h2ps")
                            for j in range(5):
                                i20 = jj * 5 + j
                                i5 = i20 // 4
                                qc = qcb * 5 + i5
                                q4 = i20 % 4
                                nc.tensor.matmul(
                                    h2ps[:, j, :],
                                    sis[q4][:, i5, :],
                                    h1rs[q4][:, qc, :],
                                    start=True, stop=True)
                            nc.any.tensor_copy(
                                h2sb[:, qcb * 20 + jj * 5:
                                     qcb * 20 + (jj + 1) * 5, :],
                                h2ps[:, :, :])

                    # LN#2 stats (over o,d per query)
                    h2d = psm.tile([128, QI], F32, tag="h2d")
                    nc.vector.tensor_reduce(h2d[:, :].unsqueeze(2),
                                            h2sb[:, :, :], AX.X, AL.add)
                    sqd2 = psm.tile([128, QI], F32, tag="sqd2")
                    for kk in range(12):
                        sl = slice(kk * 25, (kk + 1) * 25)
                        sq2 = psqp.tile([128, 25 * CG], F32, tag="sq")
                        nc.scalar.activation(
                            sq2[:, :],
                            h2sb[:, sl, :].rearrange("p a b -> p (a b)"),
                            AF.Square)
                        nc.vector.tensor_reduce(
                            sqd2[:, sl].unsqueeze(2),
                            sq2[:, :].rearrange("p (a b) -> p a b", b=CG),
                            AX.X, AL.add)
                    s1q = psms.tile([128, QI], F32, tag="pmm")
                    nc.tensor.matmul(s1q[:1, :], onesc_f[:, :], h2d[:, :],
                                     start=True, stop=True)
                    s2q = psms.tile([128, QI], F32, tag="pmm")
                    nc.tensor.matmul(s2q[:1, :], onesc_f[:, :], sqd2[:, :],
                                     start=True, stop=True)
                    mu2 = psm.tile([1, QI], F32, tag="mu2")
                    nc.any.tensor_scalar(mu2[:, :], s1q[:1, :], 1.0 / 8192.0,
                                         None, AL.mult)
                    ex2 = psm.tile([1, QI], F32, tag="ex2")
                    nc.any.tensor_scalar(ex2[:, :], s2q[:1, :], 1.0 / 8192.0,
                                         None, AL.mult)
                    var2 = psm.tile([1, QI], F32, tag="var2")
                    nc.vector.tensor_tensor(var2[:, :], mu2[:, :], mu2[:, :],
                                            AL.mult)
                    nc.vector.tensor_tensor(var2[:, :], ex2[:, :], var2[:, :],
                                            AL.subtract)
                    r2 = psm.tile([1, QI], F32, tag="r2")
                    nc.any.tensor_scalar(var2[:, :], var2[:, :], 1e-5,
                                         None, AL.add)
                    nc.scalar.activation(r2[:, :], var2[:, :], AF.Sqrt)
                    nc.vector.reciprocal(r2[:, :], r2[:, :])
                    m2e = psms.tile([128, QI], F32, tag="pmm")
                    nc.tensor.matmul(m2e[:, :], ones_f[:, :], mu2[:, :],
                                     start=True, stop=True)
                    mu2e = psm.tile([128, QI], F32, tag="mu2e")
                    nc.any.tensor_copy(mu2e[:, :], m2e[:, :])
                    r2ep = psms.tile([128, QI], F32, tag="pmm")
                    nc.tensor.matmul(r2ep[:, :], ones_f[:, :], r2[:, :],
                                     start=True, stop=True)
                    r2e = psm.tile([128, QI], F32, tag="r2e")
                    nc.any.tensor_copy(r2e[:, :], r2ep[:, :])

                    # h2r = relu(h2 - mu2) in place
                    nc.vector.tensor_tensor(
                        h2sb[:, :, :], h2sb[:, :, :],
                        mu2e[:, :].unsqueeze(2).to_broadcast([128, QI, CG]),
                        AL.subtract)
                    nc.any.tensor_scalar(
                        h2sb[:, :, :].rearrange("p a b -> p (a b)"),
                        h2sb[:, :, :].rearrange("p a b -> p (a b)"),
                        0.0, None, AL.max)

                    # ---- Phase D: projection, contract over o with PSUM
                    # accumulation over d; h2sb stays in SBUF ----
                    pr0 = pso.tile([128, QI], F32, tag="pr0")
                    pr1 = pso.tile([128, QI], F32, tag="pr1")
                    prps = [pr0, pr1]
                    for d in range(CG):
                        ow = pdw.tile([128, D], BF16, tag="ow")
                        nc.sync.dma_start(ow[:, :], opw_dec[d])
                        for dh in range(2):
                            nc.tensor.matmul(
                                prps[dh][:, :],
                                ow[:, dh * 128:(dh + 1) * 128],
                                h2sb[:, :, d],
                                start=(d == 0), stop=(d == CG - 1))
                    for dh in range(2):
                        osb = pout.tile([128, QI], BF16, tag="osb")
                        nc.vector.tensor_tensor(
                            osb[:, :], prps[dh][:, :], r2e[:, :], AL.mult)
                        nc.sync.dma_start(
                            out_ext[dh, :, qoff:qoff + QI], osb[:, :])
    nc.compile()
    _CACHE["nc"] = nc
    return nc


def _host_prep(feats, query_feat, query_roi, off_w, off_b, pg_w, pg_b, op_w):
    """Vectorized numpy: addressing metadata + per-core input tensors."""
    qf = query_feat.astype(np.float32)
    offset = (qf @ off_w + off_b).reshape(B, N, G * PIN, 3)
    roi_cc = query_roi[..., :2]
    scale = 2.0 ** query_roi[..., 2:3]
    ratio = 2.0 ** np.concatenate(
        [query_roi[..., 3:4] * -0.5, query_roi[..., 3:4] * 0.5], axis=-1)
    roi_wh = scale * ratio
    sample_xy = roi_cc[:, :, None, :] + offset[..., :2] * roi_wh[:, :, None, :]
    sample_z = query_roi[..., 2:3] + offset[..., 2]
    lvl = np.arange(4, dtype=np.float32)
    logits = -((sample_z - MAP_STRIDE)[..., None] - lvl) ** 2 / TAU
    logits -= logits.max(-1, keepdims=True)
    e = np.exp(logits)
    lw = (e / e.sum(-1, keepdims=True)).astype(np.float32)  # [B,N,G*PIN,4]
    sx = sample_xy[..., 0]                                  # [B,N,G*PIN]
    sy = sample_xy[..., 1]

    # per (lvl, ycorner): pair base index + 2 slot weights, [B, N, G*PIN]
    idx_all = np.zeros((4, 2, B, N, G * PIN), np.int16)
    w_all = np.zeros((4, 2, 2, B, N, G * PIN), np.float32)
    for li, ((H, W), stride) in enumerate(zip(SIZES, STRIDES)):
        px = sx / stride - 0.5
        py = sy / stride - 0.5
        x0 = np.floor(px)
        y0 = np.floor(py)
        wx1 = (px - x0).astype(np.float32)
        wy1 = (py - y0).astype(np.float32)
        vx0 = (x0 >= 0) & (x0 <= W - 1)
        vx1 = (x0 + 1 >= 0) & (x0 + 1 <= W - 1)
        xc = np.clip(x0, -1, W - 1)
        for dy in range(2):
            yd = y0 + dy
            vy = (yd >= 0) & (yd <= H - 1)
            yc = np.clip(yd, 0, H - 1)
            wy = (wy1 if dy else 1.0 - wy1) * lw[..., li]
            idx_all[li, dy] = (yc * W + xc + 1).astype(np.int16)
            w_all[li, dy, 0] = (1.0 - wx1) * wy * (vx0 & vy)
            w_all[li, dy, 1] = wx1 * wy * (vx1 & vy)

    ew = np.zeros((64, 2), np.float32)
    for p in range(64):
        ew[p, p // 32] = 1.0

    # S column order: device col 4096 + p*128 + o holds S[o, p]
    p_idx = np.repeat(np.arange(PIN), 128)
    o_idx = np.tile(np.arange(128), PIN)
    scol = 4096 + o_idx * PIN + p_idx
    cols = np.concatenate([np.arange(4096), scol])

    qft_pk, s_qft = _pack10(
        np.ascontiguousarray(qf.reshape(QT, D).T.reshape(2, 128, QT)),
        plane_axis=1)

    in_maps = []
    for g in range(G):
        # features: flat [pad, img0 lvls, img1..., pad] x 64ch, 10-bit packed
        fparts = [np.zeros((1, CG), np.float32)]
        for b in range(B):
            for li, (H, W) in enumerate(SIZES):
                f = feats[li][b, g * CG:(g + 1) * CG]      # [64, H, W]
                fparts.append(f.reshape(CG, H * W).T)
        fcat = np.concatenate(fparts, axis=0)
        pad_tail = PIXPAD - fcat.shape[0]
        fcat = np.concatenate(
            [fcat, np.zeros((pad_tail, CG), np.float32)], axis=0)
        fb_pk, s_fb = _pack9(fcat.reshape(128, FLATC), plane_axis=0)

        idx_cols = np.empty((16, NGRP * 600), np.int16)
        cw_cols = np.empty((128, NGRP * 150), np.float32)
        for b in range(B):
            for li in range(4):
                for dy in range(2):
                    ci = b * 8 + li * 2 + dy
                    ia = idx_all[li, dy, b, :, g * PIN:(g + 1) * PIN]
                    idx_cols[:, ci * 600:(ci + 1) * 600] = \
                        ia.reshape(NSMP).reshape(600, 16).T
                    wp = np.stack(
                        [w_all[li, dy, s, b, :, g * PIN:(g + 1) * PIN]
                         .reshape(NSMP) for s in range(2)], axis=-1)
                    cw_cols[:, ci * 150:(ci + 1) * 150] = \
                        wp.reshape(75, 128, 2).transpose(1, 0, 2).reshape(
                            128, 150)
        cw_pk, s_cw = _pack10(cw_cols, plane_axis=0)

        pgw_c = pg_w[:, g * 8192:(g + 1) * 8192][:, cols].astype(
            np.float32).reshape(2, 128, 8192)
        pgw_pk, s_pgw = _pack9(pgw_c, plane_axis=1)
        pgb_c = (pg_b[g * 8192:(g + 1) * 8192][cols] * (s_pgw * s_qft)
                 ).astype(BF)[None, :]
        scl_c = np.stack([
            np.full(128, 1.0 / (s_pgw * s_qft), np.float32),
            np.full(128, 1.0 / (s_cw * s_fb), np.float32)], axis=1)
        opw_c = op_w[g * 8192:(g + 1) * 8192, :].reshape(128, CG, D)\
            .transpose(1, 0, 2).astype(np.float32)
        opw_pk, s_opw = _pack9(opw_c, plane_axis=1)
        in_maps.append({
            "fb": fb_pk, "idx": np.ascontiguousarray(idx_cols),
            "cw": cw_pk,
            "pgw": pgw_pk, "pgb": np.ascontiguousarray(pgb_c),
            "qft": qft_pk, "opw": opw_pk, "scl": scl_c,
            "e2": ew, "e2t": np.ascontiguousarray(ew.T),
            "_host_s_opw": s_opw,      # host-only: not a declared input
        })
    return in_maps


def kernel(feat0, feat1, feat2, feat3, query_feat, query_roi,
           off_w, off_b, pg_w, pg_b, op_w, op_b, ln_g, ln_b):
    feats = [np.asarray(f, np.float32) for f in (feat0, feat1, feat2, feat3)]
    query_feat = np.asarray(query_feat, np.float32)
    query_roi = np.asarray(query_roi, np.float32)
    in_maps = _host_prep(feats, query_feat, query_roi,
                         np.asarray(off_w, np.float32),
                         np.asarray(off_b, np.float32),
                         np.asarray(pg_w, np.float32),
                         np.asarray(pg_b, np.float32),
                         np.asarray(op_w, np.float32))
    nc = _build()
    cores = list(range(G))
    if not _CACHE.get("warm"):
        # first call compiles/loads the PJRT executable; run it once so the
        # steady-state call below reflects transfer+execute only
        run_bass_kernel_spmd(nc, in_maps, core_ids=cores)
        _CACHE["warm"] = True
    res = run_bass_kernel_spmd(nc, in_maps, core_ids=cores)
    outs = res.results

    op_b = np.asarray(op_b, np.float32)
    ln_g = np.asarray(ln_g, np.float32)
    ln_b = np.asarray(ln_b, np.float32)
    acc = np.zeros((D, QT), np.float32)
    for g in range(G):
        o = outs[g]
        o = o["out"] if isinstance(o, dict) else o[0]
        acc += np.asarray(o, np.float32).reshape(D, QT) / \
            in_maps[g]["_host_s_opw"]
    h = acc.T.reshape(B, N, D) + query_feat + op_b
    mu = h.mean(-1, keepdims=True)
    var = ((h - mu) ** 2).mean(-1, keepdims=True)
    return (h - mu) / np.sqrt(var + 1e-5) * ln_g + ln_b


# revision 45
# speedup vs baseline: 1.1820x; 1.0045x over previous
"""AdaptiveSamplingMixing — Trainium2 SPMD kernel (4 cores, group-parallel).

Measurement reality on this setup: the axon tunnel moves host<->device data at
~50 MB/s while the device kernel itself executes in ~2 ms/core, so the
end-to-end run_bass_kernel_spmd call is dominated by input upload.  The kernel
is organised to minimise uploaded bytes (123 MB baseline -> ~58 MB):

  - core = g (one sampling group per core, all 4 images).  The previous
    (image-pair x group) split duplicated pg_w/op_w across image pairs
    (+33 MB); group-sharding uploads every weight byte exactly once.
  - features, pg_w and op_w upload as absmax-scaled 9-bit fixed point
    (8 values in 9 u8 planes); cw and qft as 10-bit (4 values in 5 planes).
    The vector engine decodes (shift/and/scale) on device.  The feature
    descale folds into the bilinear weights, pg_w's into the params-GEMM
    psum evacuation, op_w's into the host-side group sum, so dequantisation
    costs no extra device passes.  End-to-end error ~1.1e-2 (vs ~5.2e-3 for
    all-bf16) against the 2e-2 gate; the numpy emulation of this pipeline
    predicts hardware error to ~1e-4, so the bit widths were chosen from a
    measured error/bytes sweep (fp8 fails at 2-5e-2: coherent-sum effect).
    Touched-pixel feature compaction was evaluated and shelved: the touched
    fraction swings 27%..75% across same-distribution PRNG draws, so a safe
    capacity erases the saving.
  - bilinear x-corner PAIRS are fetched with a single 512 B dma_gather
    descriptor via an overlapping strided view of the f32 feature buffer
    (elem_step=64 elems = 256 B, elem_size=128), halving gather descriptors
    and halving the uploaded index tables.
  - the output projection contracts over the POUT partition axis with PSUM
    accumulation over d, keeping h2 in SBUF (no DRAM round-trip / transposing
    regather).
  - partial outputs download as bf16.
  - kernel() runs the spmd call twice (first call warms the jax/PJRT
    executable cache); the steady-state call is what a benchmark observes.

Numerics: rsqrt factors of both inner layernorms are folded out algebraically
(LN2(r*X) == LN2(X) for per-query scales); mix2's r2 is applied as a per-query
column scale after the projection matmul.  The host computes addressing
metadata (sample indices / bilinear weights), packs/reshapes inputs, and
finishes with the 4-way group partial-sum + residual + final affine LayerNorm.
"""
import sys
sys.path.insert(0, "/opt/trn_rl_repo")
import numpy as np
import ml_dtypes

import concourse.bass as bass
import concourse.mybir as mybir
import concourse.tile as tile
from concourse import bacc
from concourse.ap import AP
from concourse.bass_utils import run_bass_kernel_spmd
from concourse.masks import make_identity

F32 = mybir.dt.float32
BF16 = mybir.dt.bfloat16
I16 = mybir.dt.int16
U8 = mybir.dt.uint8
AL = mybir.AluOpType
AF = mybir.ActivationFunctionType
AX = mybir.AxisListType
BF = ml_dtypes.bfloat16

B, N, D = 4, 300, 256
G, PIN, POUT = 4, 32, 128
CG = D // G  # 64
STRIDES = (8, 16, 32, 64)
SIZES = ((100, 160), (50, 80), (25, 40), (13, 20))
TAU, MAP_STRIDE = 2.0, 3.0

QI = 300                   # queries per image
QT = B * QI                # 1200 queries per core (all images)
PIX_IMG = sum(h * w for h, w in SIZES)           # 21260 pixels per image
LVL_BASE = (0, 16000, 20000, 21000)
LVL_ROWS = (16000, 4000, 1000, 260)
PIXPAD = 128 * 665         # 85120 >= 1 + 4*21260 + 1 (lead/tail pads)
FLATC = PIXPAD * CG // 128                       # 42560 flat cols of fb
NSMP = QI * PIN            # 9600 gather indices per (img, lvl, ycorner)
NGRP = 32                  # gather groups per core: img(4) x lvl(4) x y(2)

_CACHE = {}


def _pack10(x, plane_axis):
    """absmax-scaled signed 10-bit fixed point; 4 elems -> 5 u8 planes.

    Packs along the last axis (length divisible by 4).  Returns (planes u8
    with a new length-5 axis at plane_axis, scale s) where
    dequant = (int10 + 512 offset removed) / s.
    """
    x = np.asarray(x, np.float32)
    amax = float(np.abs(x).max())
    if not np.isfinite(amax) or amax == 0.0:
        amax = 1.0
    s = 511.0 / amax
    q = (np.clip(np.round(x * s), -512, 511).astype(np.int16) + 512).astype(
        np.uint16)
    q = q.reshape(*x.shape[:-1], x.shape[-1] // 4, 4)
    L = (q & 255).astype(np.uint8)
    H = ((q[..., 0] >> 8) | ((q[..., 1] >> 8) << 2) |
         ((q[..., 2] >> 8) << 4) | ((q[..., 3] >> 8) << 6)).astype(np.uint8)
    planes = np.stack([L[..., 0], L[..., 1], L[..., 2], L[..., 3], H],
                      axis=plane_axis)
    return np.ascontiguousarray(planes), s


def _pack9(x, plane_axis):
    """absmax-scaled signed 9-bit fixed point; 8 elems -> 9 u8 planes."""
    x = np.asarray(x, np.float32)
    amax = float(np.abs(x).max())
    if not np.isfinite(amax) or amax == 0.0:
        amax = 1.0
    s = 255.0 / amax
    q = (np.clip(np.round(x * s), -256, 255).astype(np.int16) + 256).astype(
        np.uint16)
    q = q.reshape(*x.shape[:-1], x.shape[-1] // 8, 8)
    L = (q & 255).astype(np.uint8)
    H = np.zeros(q.shape[:-1], np.uint8)
    for i in range(8):
        H |= ((q[..., i] >> 8) << i).astype(np.uint8)
    planes = np.stack([L[..., i] for i in range(8)] + [H], axis=plane_axis)
    return np.ascontiguousarray(planes), s


def _dec9(nc, pool, planes, out3, c, tagp=""):
    """device decode: planes [128,9,c] u8 -> out3 [128,c,8] float view."""
    for i in range(8):
        hi8 = pool.tile([128, c], U8, tag=f"hi8{tagp}")
        if i == 0:
            nc.vector.tensor_scalar(hi8[:, :], planes[:, 8, :], 1, None,
                                    AL.bitwise_and)
        else:
            nc.vector.tensor_scalar(hi8[:, :], planes[:, 8, :], i, None,
                                    AL.logical_shift_right)
            nc.vector.tensor_scalar(hi8[:, :], hi8[:, :], 1, None,
                                    AL.bitwise_and)
        hif = pool.tile([128, c], F32, tag=f"hif{tagp}")
        nc.vector.tensor_copy(hif[:, :], hi8[:, :])
        lof = pool.tile([128, c], F32, tag=f"lof{tagp}")
        nc.vector.tensor_copy(lof[:, :], planes[:, i, :])
        nc.vector.tensor_scalar(hif[:, :], hif[:, :], 256.0, None, AL.mult)
        nc.vector.tensor_tensor(hif[:, :], hif[:, :], lof[:, :], AL.add)
        nc.vector.tensor_scalar(out3[:, :, i], hif[:, :], 256.0, None,
                                AL.subtract)


def _dec10(nc, pool, planes, out3, c, tagp=""):
    """device decode: planes [128,5,c] u8 -> out3 [128,c,4] (f32/bf16 view),
    values = int10 (offset removed), i.e. true*scale."""
    AL_ = AL
    for i in range(4):
        hi8 = pool.tile([128, c], U8, tag=f"hi8{tagp}")
        if i == 0:
            nc.vector.tensor_scalar(hi8[:, :], planes[:, 4, :], 3, None,
                                    AL_.bitwise_and)
        else:
            nc.vector.tensor_scalar(hi8[:, :], planes[:, 4, :], 2 * i,
                                    None, AL_.logical_shift_right)
            nc.vector.tensor_scalar(hi8[:, :], hi8[:, :], 3, None,
                                    AL_.bitwise_and)
        hif = pool.tile([128, c], F32, tag=f"hif{tagp}")
        nc.vector.tensor_copy(hif[:, :], hi8[:, :])
        lof = pool.tile([128, c], F32, tag=f"lof{tagp}")
        nc.vector.tensor_copy(lof[:, :], planes[:, i, :])
        nc.vector.tensor_scalar(hif[:, :], hif[:, :], 256.0, None, AL_.mult)
        nc.vector.tensor_tensor(hif[:, :], hif[:, :], lof[:, :], AL_.add)
        nc.vector.tensor_scalar(out3[:, :, i], hif[:, :], 512.0, None,
                                AL_.subtract)


def _build():
    if "nc" in _CACHE:
        return _CACHE["nc"]
    nc = bacc.Bacc(None, target_bir_lowering=False, debug=False)

    fb_pk = nc.declare_dram_parameter("fb", [9, 128, FLATC // 8], U8,
                                      isOutput=False)
    idx_in = nc.declare_dram_parameter("idx", [16, NGRP * 600], I16,
                                       isOutput=False)
    cw_pk = nc.declare_dram_parameter("cw", [5, 128, NGRP * 150 // 4], U8,
                                      isOutput=False)
    pgw_pk = nc.declare_dram_parameter("pgw", [2, 9, 128, 1024], U8,
                                       isOutput=False)
    pgb_in = nc.declare_dram_parameter("pgb", [1, 8192], BF16, isOutput=False)
    qft_pk = nc.declare_dram_parameter("qft", [2, 5, 128, QT // 4], U8,
                                       isOutput=False)
    opw_pk = nc.declare_dram_parameter("opw", [CG, 9, 128, D // 8], U8,
                                       isOutput=False)
    scl_in = nc.declare_dram_parameter("scl", [128, 2], F32, isOutput=False)
    e2_in = nc.declare_dram_parameter("e2", [64, 2], F32, isOutput=False)
    e2t_in = nc.declare_dram_parameter("e2t", [2, 64], F32, isOutput=False)
    out_ext = nc.declare_dram_parameter("out", [2, 128, QT], BF16,
                                        isOutput=True)

    with tile.TileContext(nc) as tc:
        with (
            tc.tile_pool(name="dram", bufs=1, space="DRAM") as dp,
            tc.tile_pool(name="const", bufs=1) as cp,
        ):
            fs32 = dp.tile([PIXPAD, CG], F32, tag="fs32")
            pdram = dp.tile([QT, 8192], BF16, tag="pdram")
            opw_dec = dp.tile([CG, 128, D], BF16, tag="opw_dec")

            ident = cp.tile([128, 128], F32, tag="ident")
            make_identity(nc, ident[:, :])
            e2 = cp.tile([64, 2], F32, tag="e2")
            nc.sync.dma_start(e2[:, :], e2_in[:, :])
            e2t = cp.tile([2, 64], F32, tag="e2t")
            nc.sync.dma_start(e2t[:, :], e2t_in[:, :])
            ones_f = cp.tile([1, 128], F32, tag="ones_f")
            nc.vector.memset(ones_f[:, :], 1.0)
            ones_b = cp.tile([1, 128], BF16, tag="ones_b")
            nc.vector.memset(ones_b[:, :], 1.0)
            onesc_f = cp.tile([128, 1], F32, tag="onesc_f")
            nc.vector.memset(onesc_f[:, :], 1.0)
            cw = cp.tile([128, NGRP * 150], BF16, tag="cw")
            scl = cp.tile([128, 2], F32, tag="scl")
            nc.sync.dma_start(scl[:, :], scl_in[:, :])
            # decode 10-bit cw planes, fold in 1/(s_cw*s_fb)
            with tc.tile_pool(name="pcw", bufs=1) as pcw:
                NQC = NGRP * 150 // 4
                plc = pcw.tile([128, 5, NQC], U8, tag="plc")
                for i in range(5):
                    nc.sync.dma_start(plc[:, i, :], cw_pk[i])
                cwf = pcw.tile([128, NQC, 4], F32, tag="cwf")
                _dec10(nc, pcw, plc, cwf, NQC, tagp="c")
                nc.vector.tensor_tensor(
                    cw[:, :].rearrange("p (a b) -> p a b", b=4),
                    cwf[:, :, :],
                    scl[:, 1:2].unsqueeze(2).to_broadcast([128, NQC, 4]),
                    AL.mult)

            # ---- Phase A: decode 9-bit features to f32 gather source ----
            fs_flat = fs32[:, :].rearrange("r c -> (r c)").rearrange(
                "(p i) -> p i", p=128)
            fs_flat3 = fs_flat.rearrange("p (a b) -> p a b", b=8)
            with tc.tile_pool(name="pa", bufs=2) as pa:
                NO = FLATC // 8          # 5320 octets per partition
                CH = NO // 8
                for ch in range(8):
                    sl = slice(ch * CH, (ch + 1) * CH)
                    pl = pa.tile([128, 9, CH], U8, tag="pl")
                    for i in range(9):
                        nc.sync.dma_start(pl[:, i, :], fb_pk[i][:, sl])
                    t32 = pa.tile([128, CH, 8], F32, tag="t32")
                    _dec9(nc, pa, pl, t32, CH, tagp="a")
                    nc.sync.dma_start(fs_flat3[:, sl, :], t32[:, :, :])

            # ---- opw decode prologue: u8 planes -> bf16 in DRAM ----
            with tc.tile_pool(name="pod", bufs=2) as pod:
                for d0 in range(0, CG, 4):
                    pl = pod.tile([128, 9, 4 * (D // 8)], U8, tag="plo")
                    for i in range(9):
                        nc.sync.dma_start(
                            pl[:, i, :].rearrange("p (d c) -> p d c", d=4),
                            opw_pk[d0:d0 + 4, i].rearrange("d p c -> p d c"))
                    owf = pod.tile([128, 4 * (D // 8), 8], BF16, tag="owf")
                    _dec9(nc, pod, pl, owf, 4 * (D // 8), tagp="o")
                    nc.sync.dma_start(
                        opw_dec[d0:d0 + 4].rearrange("d p c -> p d c"),
                        owf[:, :, :].rearrange("p (d c) f -> p d (c f)", d=4))

            # ---- Phase B: params GEMM -> pdram [QT, 8192] (q-major, bf16) --
            with (
                tc.tile_pool(name="pb", bufs=2) as pb,
                tc.tile_pool(name="pbw", bufs=1) as pbw,
                tc.tile_pool(name="psb", bufs=4, space="PSUM") as psb,
            ):
                pgw_sb = []
                for k in range(2):
                    w = pbw.tile([128, 8192], BF16, tag=f"pgw{k}")
                    w3 = w[:, :].rearrange("p (a b) -> p a b", b=8)
                    for ch in range(4):
                        sl = slice(ch * 256, (ch + 1) * 256)
                        pl = pb.tile([128, 9, 256], U8, tag="plw")
                        for i in range(9):
                            nc.sync.dma_start(pl[:, i, :], pgw_pk[k, i][:, sl])
                        _dec9(nc, pb, pl, w3[:, sl, :], 256, tagp="w")
                    pgw_sb.append(w)
                pgb_sb = pbw.tile([1, 8192], BF16, tag="pgb")
                nc.sync.dma_start(pgb_sb[:, :], pgb_in[:, :])
                qft_sb = []
                for k in range(2):
                    w = pbw.tile([128, QT], BF16, tag=f"qft{k}")
                    plq = pb.tile([128, 5, QT // 4], U8, tag="plq")
                    for i in range(5):
                        nc.sync.dma_start(plq[:, i, :], qft_pk[k, i])
                    _dec10(nc, pb,
                           plq, w[:, :].rearrange("p (a b) -> p a b", b=4),
                           QT // 4, tagp="q")
                    qft_sb.append(w)
                for qb in range(10):
                    qs = slice(qb * 120, (qb + 1) * 120)
                    qsb = pb.tile([120, 8192], BF16, tag="qsb")
                    for cb in range(16):
                        cs = slice(cb * 512, (cb + 1) * 512)
                        ps = psb.tile([120, 512], F32, tag="ps")
                        nc.tensor.matmul(ps[:, :], qft_sb[0][:, qs],
                                         pgw_sb[0][:, cs], start=True,
                                         stop=False)
                        nc.tensor.matmul(ps[:, :], qft_sb[1][:, qs],
                                         pgw_sb[1][:, cs], start=False,
                                         stop=False)
                        nc.tensor.matmul(ps[:, :], ones_b[0:1, :120],
                                         pgb_sb[0:1, cs], start=False,
                                         stop=True)
                        # evac with 1/s_pgw descale (scale is a per-core input)
                        nc.vector.tensor_tensor(
                            qsb[:, cs], ps[:, :],
                            scl[:120, 0:1].to_broadcast([120, 512]), AL.mult)
                    nc.sync.dma_start(pdram[qs, :], qsb[:, :])

            # ---- Phase C/D per image ----
            from contextlib import ExitStack
            with ExitStack() as stack:
                pool = lambda n, b, **kw: stack.enter_context(
                    tc.tile_pool(name=n, bufs=b, **kw))
                pidx = pool("pidx", 1)
                pg = pool("pg", 1)
                pacc = pool("pacc", 1)
                pms = pool("pms", 2)
                pstp = pool("pst", 3)
                ph = pool("ph", 1)
                psqp = pool("psq", 1)
                psm = pool("psmall", 2)
                pdw = pool("pdw", 3)
                pout = pool("pout", 2)
                psc = pool("psc", 1, space="PSUM")
                psh2 = pool("psh2", 2, space="PSUM")
                psc2 = pool("psc2", 1, space="PSUM")
                psms = pool("psms", 1, space="PSUM")
                pso = pool("pso", 1, space="PSUM")
                for img in range(B):
                    qoff = img * QI
                    idx_sb = pidx.tile([128, 8 * 600], I16, tag="idx")
                    for r in range(8):
                        nc.sync.dma_start(
                            idx_sb[r * 16:(r + 1) * 16, :],
                            idx_in[:, img * 4800:(img + 1) * 4800])

                    acc = pacc.tile([128, 75, CG], F32, tag="acc")
                    for grp in range(8):
                        li, dy = grp // 2, grp % 2
                        a = img * PIX_IMG + LVL_BASE[li]
                        rows = LVL_ROWS[li] + 2
                        base_ap = fs32[a:a + rows, :]
                        pair = AP(base_ap.tensor, base_ap.offset,
                                  [(CG, rows), (1, 2 * CG)])
                        ci = img * 8 + grp
                        for half in range(2):
                            c0 = 40 * half
                            ncol = 40 if half == 0 else 35
                            v = pg.tile([128, 40, 128], F32, tag="v")
                            for cc in range(5):
                                gc = 5 * half + cc
                                nn = 1024 if gc < 9 else 384
                                nc.gpsimd.dma_gather(
                                    v[:, cc * 8:cc * 8 + nn // 128, :],
                                    pair,
                                    idx_sb[:, grp * 600 + gc * 64:
                                           grp * 600 + gc * 64 + nn // 16],
                                    nn, nn, 2 * CG, elem_step=CG)
                            v4 = v[:, 0:ncol, :].rearrange(
                                "p a (s c) -> p a s c", s=2)
                            wexp = cw[:, ci * 150 + c0 * 2:
                                      ci * 150 + (c0 + ncol) * 2].rearrange(
                                "p (a s) -> p a s", s=2).unsqueeze(
                                3).to_broadcast([128, ncol, 2, CG])
                            nc.vector.tensor_tensor(v4, v4, wexp, AL.mult)
                            sl = acc[:, c0:c0 + ncol, :]
                            if grp == 0:
                                nc.vector.tensor_tensor(
                                    sl, v[:, 0:ncol, 0:CG],
                                    v[:, 0:ncol, CG:], AL.add)
                            else:
                                nc.vector.tensor_tensor(
                                    sl, sl, v[:, 0:ncol, 0:CG], AL.add)
                                nc.vector.tensor_tensor(
                                    sl, sl, v[:, 0:ncol, CG:], AL.add)

                    # ---- mix1: per-query sampled @ M (queries 4-stacked on
                    # partitions by the gather layout) ----
                    h1A = ph.tile([CG, 75, CG], BF16, tag="h1A")
                    h1B = ph.tile([CG, 75, CG], BF16, tag="h1B")
                    for qcb in range(15):
                        mi = pms.tile([CG, 20, CG], BF16, tag="mi")
                        nc.sync.dma_start(
                            mi[:, :, :],
                            pdram[qoff + qcb * 20:qoff + (qcb + 1) * 20,
                                  0:4096].rearrange("i (c d) -> c i d", c=CG))
                        h1psA = psc.tile([CG, 5, CG], F32, tag="h1psA")
                        h1psB = psc.tile([CG, 5, CG], F32, tag="h1psB")
                        for j in range(5):
                            qc = qcb * 5 + j
                            pst = psc2.tile([CG, 128], F32, tag="pst")
                            nc.tensor.transpose(pst[:, :], acc[:, qc, :],
                                                ident[:, :])
                            sT = pstp.tile([CG, 128], BF16, tag="sT")
                            nc.any.tensor_copy(sT[:, :], pst[:, :])
                            for q4 in range(4):
                                hp = h1psA if q4 < 2 else h1psB
                                pb_ = (q4 % 2) * PIN
                                nc.tensor.matmul(
                                    hp[pb_:pb_ + PIN, j, :],
                                    sT[:, q4 * PIN:(q4 + 1) * PIN],
                                    mi[:, j * 4 + q4, :],
                                    start=True, stop=True)
                        nc.any.tensor_copy(h1A[:, qcb * 5:(qcb + 1) * 5, :],
                                           h1psA[:, :, :])
                        nc.any.tensor_copy(h1B[:, qcb * 5:(qcb + 1) * 5, :],
                                           h1psB[:, :, :])

                    # LN#1: mean-center per query (rsqrt folded out), relu
                    h1rs = []
                    for hi, h1h in enumerate((h1A, h1B)):
                        h1d = psm.tile([CG, 75], F32, tag="h1d")
                        nc.vector.tensor_reduce(h1d[:, :].unsqueeze(2),
                                                h1h[:, :, :], AX.X, AL.add)
                        s1p = psms.tile([128, QI], F32, tag="pmm")
                        nc.tensor.matmul(s1p[:2, :75], e2[:, :], h1d[:, :],
                                         start=True, stop=True)
                        mu1 = psm.tile([2, 75], F32, tag="mu1")
                        nc.any.tensor_scalar(mu1[:, :], s1p[:2, :75],
                                             1.0 / 2048.0, None, AL.mult)
                        m1e = psms.tile([128, QI], F32, tag="pmm")
                        nc.tensor.matmul(m1e[:CG, :75], e2t[:, :], mu1[:, :],
                                         start=True, stop=True)
                        mu1e = psm.tile([CG, 75], F32, tag="mu1e")
                        nc.any.tensor_copy(mu1e[:, :], m1e[:CG, :75])
                        for hq in range(2):
                            q4 = hi * 2 + hq
                            pb_ = hq * PIN
                            h1r = ph.tile([PIN, 75, CG], BF16,
                                          tag=f"h1rq{q4}")
                            nc.vector.tensor_tensor(
                                h1r[:, :, :], h1h[pb_:pb_ + PIN, :, :],
                                mu1e[pb_:pb_ + PIN, :].unsqueeze(
                                    2).to_broadcast([PIN, 75, CG]),
                                AL.subtract)
                            nc.any.tensor_scalar(
                                h1r[:, :, :].rearrange("p a b -> p (a b)"),
                                h1r[:, :, :].rearrange("p a b -> p (a b)"),
                                0.0, None, AL.max)
                            h1rs.append(h1r)

                    # ---- mix2: h2[q] = S_q @ h1r_q -> h2sb [128 o, 300, 64]
                    h2sb = ph.tile([128, QI, CG], BF16, tag="h2sb")
                    for qcb in range(15):
                        blk = pdram[qoff + qcb * 20:qoff + (qcb + 1) * 20,
                                    4096:8192].rearrange(
                            "(i q) (p o) -> q p i o", i=5, p=PIN)
                        sis = []
                        for q4 in range(4):
                            si = pms.tile([PIN, 5, 128], BF16,
                                          tag=f"siq{q4}")
                            nc.sync.dma_start(si[:, :, :], blk[q4])
                            sis.append(si)
                        for jj in range(4):
                            h2ps = psh2.tile([128, 5, CG], F32, tag="h2ps")
                            for j in range(5):
                                i20 = jj * 5 + j
                                i5 = i20 // 4
                                qc = qcb * 5 + i5
                                q4 = i20 % 4
                                nc.tensor.matmul(
                                    h2ps[:, j, :],
                                    sis[q4][:, i5, :],
                                    h1rs[q4][:, qc, :],
                                    start=True, stop=True)
                            nc.any.tensor_copy(
                                h2sb[:, qcb * 20 + jj * 5:
                                     qcb * 20 + (jj + 1) * 5, :],
                                h2ps[:, :, :])

                    # LN#2 stats (over o,d per query)
                    h2d = psm.tile([128, QI], F32, tag="h2d")
                    nc.vector.tensor_reduce(h2d[:, :].unsqueeze(2),
                                            h2sb[:, :, :], AX.X, AL.add)
                    sqd2 = psm.tile([128, QI], F32, tag="sqd2")
                    for kk in range(12):
                        sl = slice(kk * 25, (kk + 1) * 25)
                        sq2 = psqp.tile([128, 25 * CG], F32, tag="sq")
                        nc.scalar.activation(
                            sq2[:, :],
                            h2sb[:, sl, :].rearrange("p a b -> p (a b)"),
                            AF.Square)
                        nc.vector.tensor_reduce(
                            sqd2[:, sl].unsqueeze(2),
                            sq2[:, :].rearrange("p (a b) -> p a b", b=CG),
                            AX.X, AL.add)
                    s1q = psms.tile([128, QI], F32, tag="pmm")
                    nc.tensor.matmul(s1q[:1, :], onesc_f[:, :], h2d[:, :],
                                     start=True, stop=True)
                    s2q = psms.tile([128, QI], F32, tag="pmm")
                    nc.tensor.matmul(s2q[:1, :], onesc_f[:, :], sqd2[:, :],
                                     start=True, stop=True)
                    mu2 = psm.tile([1, QI], F32, tag="mu2")
                    nc.any.tensor_scalar(mu2[:, :], s1q[:1, :], 1.0 / 8192.0,
                                         None, AL.mult)
                    ex2 = psm.tile([1, QI], F32, tag="ex2")
                    nc.any.tensor_scalar(ex2[:, :], s2q[:1, :], 1.0 / 8192.0,
                                         None, AL.mult)
                    var2 = psm.tile([1, QI], F32, tag="var2")
                    nc.vector.tensor_tensor(var2[:, :], mu2[:, :], mu2[:, :],
                                            AL.mult)
                    nc.vector.tensor_tensor(var2[:, :], ex2[:, :], var2[:, :],
                                            AL.subtract)
                    r2 = psm.tile([1, QI], F32, tag="r2")
                    nc.any.tensor_scalar(var2[:, :], var2[:, :], 1e-5,
                                         None, AL.add)
                    nc.scalar.activation(r2[:, :], var2[:, :], AF.Sqrt)
                    nc.vector.reciprocal(r2[:, :], r2[:, :])
                    m2e = psms.tile([128, QI], F32, tag="pmm")
                    nc.tensor.matmul(m2e[:, :], ones_f[:, :], mu2[:, :],
                                     start=True, stop=True)
                    mu2e = psm.tile([128, QI], F32, tag="mu2e")
                    nc.any.tensor_copy(mu2e[:, :], m2e[:, :])
                    r2ep = psms.tile([128, QI], F32, tag="pmm")
                    nc.tensor.matmul(r2ep[:, :], ones_f[:, :], r2[:, :],
                                     start=True, stop=True)
                    r2e = psm.tile([128, QI], F32, tag="r2e")
                    nc.any.tensor_copy(r2e[:, :], r2ep[:, :])

                    # h2r = relu(h2 - mu2) in place
                    nc.vector.tensor_tensor(
                        h2sb[:, :, :], h2sb[:, :, :],
                        mu2e[:, :].unsqueeze(2).to_broadcast([128, QI, CG]),
                        AL.subtract)
                    nc.any.tensor_scalar(
                        h2sb[:, :, :].rearrange("p a b -> p (a b)"),
                        h2sb[:, :, :].rearrange("p a b -> p (a b)"),
                        0.0, None, AL.max)

                    # ---- Phase D: projection, contract over o with PSUM
                    # accumulation over d; h2sb stays in SBUF ----
                    pr0 = pso.tile([128, QI], F32, tag="pr0")
                    pr1 = pso.tile([128, QI], F32, tag="pr1")
                    prps = [pr0, pr1]
                    for d in range(CG):
                        ow = pdw.tile([128, D], BF16, tag="ow")
                        nc.sync.dma_start(ow[:, :], opw_dec[d])
                        for dh in range(2):
                            nc.tensor.matmul(
                                prps[dh][:, :],
                                ow[:, dh * 128:(dh + 1) * 128],
                                h2sb[:, :, d],
                                start=(d == 0), stop=(d == CG - 1))
                    for dh in range(2):
                        osb = pout.tile([128, QI], BF16, tag="osb")
                        nc.vector.tensor_tensor(
                            osb[:, :], prps[dh][:, :], r2e[:, :], AL.mult)
                        nc.sync.dma_start(
                            out_ext[dh, :, qoff:qoff + QI], osb[:, :])
    nc.compile()
    _CACHE["nc"] = nc
    return nc


def _host_prep(feats, query_feat, query_roi, off_w, off_b, pg_w, pg_b, op_w):
    """Vectorized numpy: addressing metadata + per-core input tensors."""
    qf = query_feat.astype(np.float32)
    offset = (qf @ off_w + off_b).reshape(B, N, G * PIN, 3)
    roi_cc = query_roi[..., :2]
    scale = 2.0 ** query_roi[..., 2:3]
    ratio = 2.0 ** np.concatenate(
        [query_roi[..., 3:4] * -0.5, query_roi[..., 3:4] * 0.5], axis=-1)
    roi_wh = scale * ratio
    sample_xy = roi_cc[:, :, None, :] + offset[..., :2] * roi_wh[:, :, None, :]
    sample_z = query_roi[..., 2:3] + offset[..., 2]
    lvl = np.arange(4, dtype=np.float32)
    logits = -((sample_z - MAP_STRIDE)[..., None] - lvl) ** 2 / TAU
    logits -= logits.max(-1, keepdims=True)
    e = np.exp(logits)
    lw = (e / e.sum(-1, keepdims=True)).astype(np.float32)  # [B,N,G*PIN,4]
    sx = sample_xy[..., 0]                                  # [B,N,G*PIN]
    sy = sample_xy[..., 1]

    # per (lvl, ycorner): pair base index + 2 slot weights, [B, N, G*PIN]
    idx_all = np.zeros((4, 2, B, N, G * PIN), np.int16)
    w_all = np.zeros((4, 2, 2, B, N, G * PIN), np.float32)
    for li, ((H, W), stride) in enumerate(zip(SIZES, STRIDES)):
        px = sx / stride - 0.5
        py = sy / stride - 0.5
        x0 = np.floor(px)
        y0 = np.floor(py)
        wx1 = (px - x0).astype(np.float32)
        wy1 = (py - y0).astype(np.float32)
        vx0 = (x0 >= 0) & (x0 <= W - 1)
        vx1 = (x0 + 1 >= 0) & (x0 + 1 <= W - 1)
        xc = np.clip(x0, -1, W - 1)
        for dy in range(2):
            yd = y0 + dy
            vy = (yd >= 0) & (yd <= H - 1)
            yc = np.clip(yd, 0, H - 1)
            wy = (wy1 if dy else 1.0 - wy1) * lw[..., li]
            idx_all[li, dy] = (yc * W + xc + 1).astype(np.int16)
            w_all[li, dy, 0] = (1.0 - wx1) * wy * (vx0 & vy)
            w_all[li, dy, 1] = wx1 * wy * (vx1 & vy)

    ew = np.zeros((64, 2), np.float32)
    for p in range(64):
        ew[p, p // 32] = 1.0

    # S column order: device col 4096 + p*128 + o holds S[o, p]
    p_idx = np.repeat(np.arange(PIN), 128)
    o_idx = np.tile(np.arange(128), PIN)
    scol = 4096 + o_idx * PIN + p_idx
    cols = np.concatenate([np.arange(4096), scol])

    qft_pk, s_qft = _pack10(
        np.ascontiguousarray(qf.reshape(QT, D).T.reshape(2, 128, QT)),
        plane_axis=1)

    in_maps = []
    for g in range(G):
        # features: flat [pad, img0 lvls, img1..., pad] x 64ch, 10-bit packed
        fparts = [np.zeros((1, CG), np.float32)]
        for b in range(B):
            for li, (H, W) in enumerate(SIZES):
                f = feats[li][b, g * CG:(g + 1) * CG]      # [64, H, W]
                fparts.append(f.reshape(CG, H * W).T)
        fcat = np.concatenate(fparts, axis=0)
        pad_tail = PIXPAD - fcat.shape[0]
        fcat = np.concatenate(
            [fcat, np.zeros((pad_tail, CG), np.float32)], axis=0)
        fb_pk, s_fb = _pack9(fcat.reshape(128, FLATC), plane_axis=0)

        idx_cols = np.empty((16, NGRP * 600), np.int16)
        cw_cols = np.empty((128, NGRP * 150), np.float32)
        for b in range(B):
            for li in range(4):
                for dy in range(2):
                    ci = b * 8 + li * 2 + dy
                    ia = idx_all[li, dy, b, :, g * PIN:(g + 1) * PIN]
                    idx_cols[:, ci * 600:(ci + 1) * 600] = \
                        ia.reshape(NSMP).reshape(600, 16).T
                    wp = np.stack(
                        [w_all[li, dy, s, b, :, g * PIN:(g + 1) * PIN]
                         .reshape(NSMP) for s in range(2)], axis=-1)
                    cw_cols[:, ci * 150:(ci + 1) * 150] = \
                        wp.reshape(75, 128, 2).transpose(1, 0, 2).reshape(
                            128, 150)
        cw_pk, s_cw = _pack10(cw_cols, plane_axis=0)

        pgw_c = pg_w[:, g * 8192:(g + 1) * 8192][:, cols].astype(
            np.float32).reshape(2, 128, 8192)
        pgw_pk, s_pgw = _pack9(pgw_c, plane_axis=1)
        pgb_c = (pg_b[g * 8192:(g + 1) * 8192][cols] * (s_pgw * s_qft)
                 ).astype(BF)[None, :]
        scl_c = np.stack([
            np.full(128, 1.0 / (s_pgw * s_qft), np.float32),
            np.full(128, 1.0 / (s_cw * s_fb), np.float32)], axis=1)
        opw_c = op_w[g * 8192:(g + 1) * 8192, :].reshape(128, CG, D)\
            .transpose(1, 0, 2).astype(np.float32)
        opw_pk, s_opw = _pack9(opw_c, plane_axis=1)
        in_maps.append({
            "fb": fb_pk, "idx": np.ascontiguousarray(idx_cols),
            "cw": cw_pk,
            "pgw": pgw_pk, "pgb": np.ascontiguousarray(pgb_c),
            "qft": qft_pk, "opw": opw_pk, "scl": scl_c,
            "e2": ew, "e2t": np.ascontiguousarray(ew.T),
            "_host_s_opw": s_opw,      # host-only: not a declared input
        })
    return in_maps


def kernel(feat0, feat1, feat2, feat3, query_feat, query_roi,
           off_w, off_b, pg_w, pg_b, op_w, op_b, ln_g, ln_b):
    feats = [np.asarray(f, np.float32) for f in (feat0, feat1, feat2, feat3)]
    query_feat = np.asarray(query_feat, np.float32)
    query_roi = np.asarray(query_roi, np.float32)
    in_maps = _host_prep(feats, query_feat, query_roi,
                         np.asarray(off_w, np.float32),
                         np.asarray(off_b, np.float32),
                         np.asarray(pg_w, np.float32),
                         np.asarray(pg_b, np.float32),
                         np.asarray(op_w, np.float32))
    nc = _build()
    cores = list(range(G))
    if not _CACHE.get("warm"):
        # first call compiles/loads the PJRT executable; run it once so the
        # steady-state call below reflects transfer+execute only
        run_bass_kernel_spmd(nc, in_maps, core_ids=cores)
        _CACHE["warm"] = True
    res = run_bass_kernel_spmd(nc, in_maps, core_ids=cores)
    outs = res.results

    op_b = np.asarray(op_b, np.float32)
    ln_g = np.asarray(ln_g, np.float32)
    ln_b = np.asarray(ln_b, np.float32)
    acc = np.zeros((D, QT), np.float32)
    for g in range(G):
        o = outs[g]
        o = o["out"] if isinstance(o, dict) else o[0]
        acc += np.asarray(o, np.float32).reshape(D, QT) / \
            in_maps[g]["_host_s_opw"]
    h = acc.T.reshape(B, N, D) + query_feat + op_b
    mu = h.mean(-1, keepdims=True)
    var = ((h - mu) ** 2).mean(-1, keepdims=True)
    return (h - mu) / np.sqrt(var + 1e-5) * ln_g + ln_b


# revision 48
# speedup vs baseline: 1.2460x; 1.0542x over previous
"""AdaptiveSamplingMixing — Trainium2 SPMD kernel (4 cores, group-parallel).

Measurement reality on this setup: the axon tunnel moves host<->device data at
~50 MB/s while the device kernel itself executes in ~2 ms/core, so the
end-to-end run_bass_kernel_spmd call is dominated by input upload.  The kernel
is organised to minimise uploaded bytes (123 MB baseline -> ~58 MB):

  - core = g (one sampling group per core, all 4 images).  The previous
    (image-pair x group) split duplicated pg_w/op_w across image pairs
    (+33 MB); group-sharding uploads every weight byte exactly once.
  - features, pg_w and op_w upload as absmax-scaled 9-bit fixed point
    (8 values in 9 u8 planes); cw and qft as 10-bit (4 values in 5 planes).
    The vector engine decodes (shift/and/scale) on device.  The feature
    descale folds into the bilinear weights, pg_w's into the params-GEMM
    psum evacuation, op_w's into the host-side group sum, so dequantisation
    costs no extra device passes.  End-to-end error ~1.1e-2 (vs ~5.2e-3 for
    all-bf16) against the 2e-2 gate; the numpy emulation of this pipeline
    predicts hardware error to ~1e-4, so the bit widths were chosen from a
    measured error/bytes sweep (fp8 fails at 2-5e-2: coherent-sum effect).
    Touched-pixel feature compaction was evaluated and shelved: the touched
    fraction swings 27%..75% across same-distribution PRNG draws, so a safe
    capacity erases the saving.
  - bilinear x-corner PAIRS are fetched with a single 512 B dma_gather
    descriptor via an overlapping strided view of the f32 feature buffer
    (elem_step=64 elems = 256 B, elem_size=128), halving gather descriptors
    and halving the uploaded index tables.
  - the output projection contracts over the POUT partition axis with PSUM
    accumulation over d, keeping h2 in SBUF (no DRAM round-trip / transposing
    regather).
  - partial outputs download as bf16.
  - kernel() runs the spmd call twice (first call warms the jax/PJRT
    executable cache); the steady-state call is what a benchmark observes.

Numerics: rsqrt factors of both inner layernorms are folded out algebraically
(LN2(r*X) == LN2(X) for per-query scales); mix2's r2 is applied as a per-query
column scale after the projection matmul.  The host computes addressing
metadata (sample indices / bilinear weights), packs/reshapes inputs, and
finishes with the 4-way group partial-sum + residual + final affine LayerNorm.
"""
import sys
sys.path.insert(0, "/opt/trn_rl_repo")
import numpy as np
import ml_dtypes

import concourse.bass as bass
import concourse.mybir as mybir
import concourse.tile as tile
from concourse import bacc
from concourse.ap import AP
from concourse.bass_utils import run_bass_kernel_spmd
from concourse.masks import make_identity

F32 = mybir.dt.float32
BF16 = mybir.dt.bfloat16
I16 = mybir.dt.int16
U8 = mybir.dt.uint8
AL = mybir.AluOpType
AF = mybir.ActivationFunctionType
AX = mybir.AxisListType
BF = ml_dtypes.bfloat16

B, N, D = 4, 300, 256
G, PIN, POUT = 4, 32, 128
CG = D // G  # 64
STRIDES = (8, 16, 32, 64)
SIZES = ((100, 160), (50, 80), (25, 40), (13, 20))
TAU, MAP_STRIDE = 2.0, 3.0

QI = 300                   # queries per image
QT = B * QI                # 1200 queries per core (all images)
PIX_IMG = sum(h * w for h, w in SIZES)           # 21260 pixels per image
LVL_BASE = (0, 16000, 20000, 21000)
LVL_ROWS = (16000, 4000, 1000, 260)
PIXPAD = 128 * 665         # 85120 >= 1 + 4*21260 + 1 (lead/tail pads)
FLATC = PIXPAD * CG // 128                       # 42560 flat cols of fb
NSMP = QI * PIN            # 9600 gather indices per (img, lvl, ycorner)
NGRP = 32                  # gather groups per core: img(4) x lvl(4) x y(2)

_CACHE = {}


def _pack10(x, plane_axis):
    """absmax-scaled signed 10-bit fixed point; 4 elems -> 5 u8 planes.

    Packs along the last axis (length divisible by 4).  Returns (planes u8
    with a new length-5 axis at plane_axis, scale s) where
    dequant = (int10 + 512 offset removed) / s.
    """
    x = np.asarray(x, np.float32)
    amax = float(np.abs(x).max())
    if not np.isfinite(amax) or amax == 0.0:
        amax = 1.0
    s = 511.0 / amax
    q = (np.clip(np.round(x * s), -512, 511).astype(np.int16) + 512).astype(
        np.uint16)
    q = q.reshape(*x.shape[:-1], x.shape[-1] // 4, 4)
    L = (q & 255).astype(np.uint8)
    H = ((q[..., 0] >> 8) | ((q[..., 1] >> 8) << 2) |
         ((q[..., 2] >> 8) << 4) | ((q[..., 3] >> 8) << 6)).astype(np.uint8)
    planes = np.stack([L[..., 0], L[..., 1], L[..., 2], L[..., 3], H],
                      axis=plane_axis)
    return np.ascontiguousarray(planes), s


def _pack9(x, plane_axis):
    """absmax-scaled signed 9-bit fixed point; 8 elems -> 9 u8 planes."""
    x = np.asarray(x, np.float32)
    amax = float(np.abs(x).max())
    if not np.isfinite(amax) or amax == 0.0:
        amax = 1.0
    s = 255.0 / amax
    q = (np.clip(np.round(x * s), -256, 255).astype(np.int16) + 256).astype(
        np.uint16)
    q = q.reshape(*x.shape[:-1], x.shape[-1] // 8, 8)
    L = (q & 255).astype(np.uint8)
    H = np.zeros(q.shape[:-1], np.uint8)
    for i in range(8):
        H |= ((q[..., i] >> 8) << i).astype(np.uint8)
    planes = np.stack([L[..., i] for i in range(8)] + [H], axis=plane_axis)
    return np.ascontiguousarray(planes), s


def _dec9(nc, pool, planes, out3, c, tagp=""):
    """device decode: planes [128,9,c] u8 -> out3 [128,c,8] float view."""
    for i in range(8):
        hi8 = pool.tile([128, c], U8, tag=f"hi8{tagp}")
        if i == 0:
            nc.vector.tensor_scalar(hi8[:, :], planes[:, 8, :], 1, None,
                                    AL.bitwise_and)
        else:
            nc.vector.tensor_scalar(hi8[:, :], planes[:, 8, :], i, None,
                                    AL.logical_shift_right)
            nc.vector.tensor_scalar(hi8[:, :], hi8[:, :], 1, None,
                                    AL.bitwise_and)
        hif = pool.tile([128, c], F32, tag=f"hif{tagp}")
        nc.vector.tensor_copy(hif[:, :], hi8[:, :])
        lof = pool.tile([128, c], F32, tag=f"lof{tagp}")
        nc.vector.tensor_copy(lof[:, :], planes[:, i, :])
        nc.vector.tensor_scalar(hif[:, :], hif[:, :], 256.0, None, AL.mult)
        nc.vector.tensor_tensor(hif[:, :], hif[:, :], lof[:, :], AL.add)
        nc.vector.tensor_scalar(out3[:, :, i], hif[:, :], 256.0, None,
                                AL.subtract)


def _dec10(nc, pool, planes, out3, c, tagp=""):
    """device decode: planes [128,5,c] u8 -> out3 [128,c,4] (f32/bf16 view),
    values = int10 (offset removed), i.e. true*scale."""
    AL_ = AL
    for i in range(4):
        hi8 = pool.tile([128, c], U8, tag=f"hi8{tagp}")
        if i == 0:
            nc.vector.tensor_scalar(hi8[:, :], planes[:, 4, :], 3, None,
                                    AL_.bitwise_and)
        else:
            nc.vector.tensor_scalar(hi8[:, :], planes[:, 4, :], 2 * i,
                                    None, AL_.logical_shift_right)
            nc.vector.tensor_scalar(hi8[:, :], hi8[:, :], 3, None,
                                    AL_.bitwise_and)
        hif = pool.tile([128, c], F32, tag=f"hif{tagp}")
        nc.vector.tensor_copy(hif[:, :], hi8[:, :])
        lof = pool.tile([128, c], F32, tag=f"lof{tagp}")
        nc.vector.tensor_copy(lof[:, :], planes[:, i, :])
        nc.vector.tensor_scalar(hif[:, :], hif[:, :], 256.0, None, AL_.mult)
        nc.vector.tensor_tensor(hif[:, :], hif[:, :], lof[:, :], AL_.add)
        nc.vector.tensor_scalar(out3[:, :, i], hif[:, :], 512.0, None,
                                AL_.subtract)


def _build():
    if "nc" in _CACHE:
        return _CACHE["nc"]
    nc = bacc.Bacc(None, target_bir_lowering=False, debug=False,
                   num_swdge_queues=4)

    fb_pk = nc.declare_dram_parameter("fb", [9, 128, FLATC // 8], U8,
                                      isOutput=False)
    idx_in = nc.declare_dram_parameter("idx", [16, NGRP * 600], I16,
                                       isOutput=False)
    cw_pk = nc.declare_dram_parameter("cw", [5, 128, NGRP * 150 // 4], U8,
                                      isOutput=False)
    pgw_pk = nc.declare_dram_parameter("pgw", [2, 9, 128, 1024], U8,
                                       isOutput=False)
    pgb_in = nc.declare_dram_parameter("pgb", [1, 8192], BF16, isOutput=False)
    qft_pk = nc.declare_dram_parameter("qft", [2, 5, 128, QT // 4], U8,
                                       isOutput=False)
    opw_pk = nc.declare_dram_parameter("opw", [CG, 9, 128, D // 8], U8,
                                       isOutput=False)
    scl_in = nc.declare_dram_parameter("scl", [128, 2], F32, isOutput=False)
    e2_in = nc.declare_dram_parameter("e2", [64, 2], F32, isOutput=False)
    e2t_in = nc.declare_dram_parameter("e2t", [2, 64], F32, isOutput=False)
    out_ext = nc.declare_dram_parameter("out", [2, 128, QT], BF16,
                                        isOutput=True)

    with tile.TileContext(nc) as tc:
        with (
            tc.tile_pool(name="dram", bufs=1, space="DRAM") as dp,
            tc.tile_pool(name="const", bufs=1) as cp,
        ):
            fs32 = dp.tile([PIXPAD, CG], F32, tag="fs32")
            pdram = dp.tile([QT, 8192], BF16, tag="pdram")
            opw_dec = dp.tile([CG, 128, D], BF16, tag="opw_dec")

            ident = cp.tile([128, 128], F32, tag="ident")
            make_identity(nc, ident[:, :])
            e2 = cp.tile([64, 2], F32, tag="e2")
            nc.sync.dma_start(e2[:, :], e2_in[:, :])
            e2t = cp.tile([2, 64], F32, tag="e2t")
            nc.sync.dma_start(e2t[:, :], e2t_in[:, :])
            ones_f = cp.tile([1, 128], F32, tag="ones_f")
            nc.vector.memset(ones_f[:, :], 1.0)
            ones_b = cp.tile([1, 128], BF16, tag="ones_b")
            nc.vector.memset(ones_b[:, :], 1.0)
            onesc_f = cp.tile([128, 1], F32, tag="onesc_f")
            nc.vector.memset(onesc_f[:, :], 1.0)
            cw = cp.tile([128, NGRP * 150], BF16, tag="cw")
            scl = cp.tile([128, 2], F32, tag="scl")
            nc.sync.dma_start(scl[:, :], scl_in[:, :])
            # decode 10-bit cw planes, fold in 1/(s_cw*s_fb)
            with tc.tile_pool(name="pcw", bufs=1) as pcw:
                NQC = NGRP * 150 // 4
                plc = pcw.tile([128, 5, NQC], U8, tag="plc")
                for i in range(5):
                    nc.sync.dma_start(plc[:, i, :], cw_pk[i])
                cwf = pcw.tile([128, NQC, 4], F32, tag="cwf")
                _dec10(nc, pcw, plc, cwf, NQC, tagp="c")
                nc.vector.tensor_tensor(
                    cw[:, :].rearrange("p (a b) -> p a b", b=4),
                    cwf[:, :, :],
                    scl[:, 1:2].unsqueeze(2).to_broadcast([128, NQC, 4]),
                    AL.mult)

            # ---- Phase A: decode 9-bit features to f32 gather source ----
            fs_flat = fs32[:, :].rearrange("r c -> (r c)").rearrange(
                "(p i) -> p i", p=128)
            fs_flat3 = fs_flat.rearrange("p (a b) -> p a b", b=8)
            with tc.tile_pool(name="pa", bufs=2) as pa:
                NO = FLATC // 8          # 5320 octets per partition
                CH = NO // 8
                for ch in range(8):
                    sl = slice(ch * CH, (ch + 1) * CH)
                    pl = pa.tile([128, 9, CH], U8, tag="pl")
                    for i in range(9):
                        nc.sync.dma_start(pl[:, i, :], fb_pk[i][:, sl])
                    t32 = pa.tile([128, CH, 8], F32, tag="t32")
                    _dec9(nc, pa, pl, t32, CH, tagp="a")
                    nc.sync.dma_start(fs_flat3[:, sl, :], t32[:, :, :])

            # ---- opw decode prologue: u8 planes -> bf16 in DRAM ----
            with tc.tile_pool(name="pod", bufs=2) as pod:
                for d0 in range(0, CG, 4):
                    pl = pod.tile([128, 9, 4 * (D // 8)], U8, tag="plo")
                    for i in range(9):
                        nc.sync.dma_start(
                            pl[:, i, :].rearrange("p (d c) -> p d c", d=4),
                            opw_pk[d0:d0 + 4, i].rearrange("d p c -> p d c"))
                    owf = pod.tile([128, 4 * (D // 8), 8], BF16, tag="owf")
                    _dec9(nc, pod, pl, owf, 4 * (D // 8), tagp="o")
                    nc.sync.dma_start(
                        opw_dec[d0:d0 + 4].rearrange("d p c -> p d c"),
                        owf[:, :, :].rearrange("p (d c) f -> p d (c f)", d=4))

            # ---- Phase B: params GEMM -> pdram [QT, 8192] (q-major, bf16) --
            with (
                tc.tile_pool(name="pb", bufs=2) as pb,
                tc.tile_pool(name="pbw", bufs=1) as pbw,
                tc.tile_pool(name="psb", bufs=4, space="PSUM") as psb,
            ):
                pgw_sb = []
                for k in range(2):
                    w = pbw.tile([128, 8192], BF16, tag=f"pgw{k}")
                    w3 = w[:, :].rearrange("p (a b) -> p a b", b=8)
                    for ch in range(4):
                        sl = slice(ch * 256, (ch + 1) * 256)
                        pl = pb.tile([128, 9, 256], U8, tag="plw")
                        for i in range(9):
                            nc.sync.dma_start(pl[:, i, :], pgw_pk[k, i][:, sl])
                        _dec9(nc, pb, pl, w3[:, sl, :], 256, tagp="w")
                    pgw_sb.append(w)
                pgb_sb = pbw.tile([1, 8192], BF16, tag="pgb")
                nc.sync.dma_start(pgb_sb[:, :], pgb_in[:, :])
                qft_sb = []
                for k in range(2):
                    w = pbw.tile([128, QT], BF16, tag=f"qft{k}")
                    plq = pb.tile([128, 5, QT // 4], U8, tag="plq")
                    for i in range(5):
                        nc.sync.dma_start(plq[:, i, :], qft_pk[k, i])
                    _dec10(nc, pb,
                           plq, w[:, :].rearrange("p (a b) -> p a b", b=4),
                           QT // 4, tagp="q")
                    qft_sb.append(w)
                for qb in range(10):
                    qs = slice(qb * 120, (qb + 1) * 120)
                    qsb = pb.tile([120, 8192], BF16, tag="qsb")
                    for cb in range(16):
                        cs = slice(cb * 512, (cb + 1) * 512)
                        ps = psb.tile([120, 512], F32, tag="ps")
                        nc.tensor.matmul(ps[:, :], qft_sb[0][:, qs],
                                         pgw_sb[0][:, cs], start=True,
                                         stop=False)
                        nc.tensor.matmul(ps[:, :], qft_sb[1][:, qs],
                                         pgw_sb[1][:, cs], start=False,
                                         stop=False)
                        nc.tensor.matmul(ps[:, :], ones_b[0:1, :120],
                                         pgb_sb[0:1, cs], start=False,
                                         stop=True)
                        # evac with 1/s_pgw descale (scale is a per-core input)
                        nc.vector.tensor_tensor(
                            qsb[:, cs], ps[:, :],
                            scl[:120, 0:1].to_broadcast([120, 512]), AL.mult)
                    nc.sync.dma_start(pdram[qs, :], qsb[:, :])

            # ---- Phase C/D per image ----
            from contextlib import ExitStack
            with ExitStack() as stack:
                pool = lambda n, b, **kw: stack.enter_context(
                    tc.tile_pool(name=n, bufs=b, **kw))
                pidx = pool("pidx", 1)
                pg = pool("pg", 1)
                pacc = pool("pacc", 1)
                pms = pool("pms", 2)
                pstp = pool("pst", 3)
                ph = pool("ph", 1)
                psqp = pool("psq", 1)
                psm = pool("psmall", 2)
                pdw = pool("pdw", 3)
                pout = pool("pout", 2)
                psc = pool("psc", 1, space="PSUM")
                psh2 = pool("psh2", 2, space="PSUM")
                psc2 = pool("psc2", 1, space="PSUM")
                psms = pool("psms", 1, space="PSUM")
                pso = pool("pso", 1, space="PSUM")
                n_gather = 0    # global gather counter: queue = n % 4 stays
                # congruent with the 8 DMASW sem lanes (8 % 4 == 0), so each
                # lane only ever sees one SWDGE queue
                for img in range(B):
                    qoff = img * QI
                    idx_sb = pidx.tile([128, 8 * 600], I16, tag="idx")
                    for r in range(8):
                        nc.sync.dma_start(
                            idx_sb[r * 16:(r + 1) * 16, :],
                            idx_in[:, img * 4800:(img + 1) * 4800])

                    acc = pacc.tile([128, 75, CG], F32, tag="acc")
                    for grp in range(8):
                        li, dy = grp // 2, grp % 2
                        a = img * PIX_IMG + LVL_BASE[li]
                        rows = LVL_ROWS[li] + 2
                        base_ap = fs32[a:a + rows, :]
                        pair = AP(base_ap.tensor, base_ap.offset,
                                  [(CG, rows), (1, 2 * CG)])
                        ci = img * 8 + grp
                        for half in range(2):
                            c0 = 40 * half
                            ncol = 40 if half == 0 else 35
                            v = pg.tile([128, 40, 128], F32, tag="v")
                            for cc in range(5):
                                gc = 5 * half + cc
                                nn = 1024 if gc < 9 else 384
                                nc.gpsimd.dma_gather(
                                    v[:, cc * 8:cc * 8 + nn // 128, :],
                                    pair,
                                    idx_sb[:, grp * 600 + gc * 64:
                                           grp * 600 + gc * 64 + nn // 16],
                                    nn, nn, 2 * CG, elem_step=CG,
                                    queue_num=n_gather % 4)
                                n_gather += 1
                            v4 = v[:, 0:ncol, :].rearrange(
                                "p a (s c) -> p a s c", s=2)
                            wexp = cw[:, ci * 150 + c0 * 2:
                                      ci * 150 + (c0 + ncol) * 2].rearrange(
                                "p (a s) -> p a s", s=2).unsqueeze(
                                3).to_broadcast([128, ncol, 2, CG])
                            nc.vector.tensor_tensor(v4, v4, wexp, AL.mult)
                            sl = acc[:, c0:c0 + ncol, :]
                            if grp == 0:
                                nc.vector.tensor_tensor(
                                    sl, v[:, 0:ncol, 0:CG],
                                    v[:, 0:ncol, CG:], AL.add)
                            else:
                                nc.vector.tensor_tensor(
                                    sl, sl, v[:, 0:ncol, 0:CG], AL.add)
                                nc.vector.tensor_tensor(
                                    sl, sl, v[:, 0:ncol, CG:], AL.add)

                    # ---- mix1: per-query sampled @ M (queries 4-stacked on
                    # partitions by the gather layout) ----
                    h1A = ph.tile([CG, 75, CG], BF16, tag="h1A")
                    h1B = ph.tile([CG, 75, CG], BF16, tag="h1B")
                    for qcb in range(15):
                        mi = pms.tile([CG, 20, CG], BF16, tag="mi")
                        nc.sync.dma_start(
                            mi[:, :, :],
                            pdram[qoff + qcb * 20:qoff + (qcb + 1) * 20,
                                  0:4096].rearrange("i (c d) -> c i d", c=CG))
                        h1psA = psc.tile([CG, 5, CG], F32, tag="h1psA")
                        h1psB = psc.tile([CG, 5, CG], F32, tag="h1psB")
                        for j in range(5):
                            qc = qcb * 5 + j
                            pst = psc2.tile([CG, 128], F32, tag="pst")
                            nc.tensor.transpose(pst[:, :], acc[:, qc, :],
                                                ident[:, :])
                            sT = pstp.tile([CG, 128], BF16, tag="sT")
                            nc.any.tensor_copy(sT[:, :], pst[:, :])
                            for q4 in range(4):
                                hp = h1psA if q4 < 2 else h1psB
                                pb_ = (q4 % 2) * PIN
                                nc.tensor.matmul(
                                    hp[pb_:pb_ + PIN, j, :],
                                    sT[:, q4 * PIN:(q4 + 1) * PIN],
                                    mi[:, j * 4 + q4, :],
                                    start=True, stop=True)
                        nc.any.tensor_copy(h1A[:, qcb * 5:(qcb + 1) * 5, :],
                                           h1psA[:, :, :])
                        nc.any.tensor_copy(h1B[:, qcb * 5:(qcb + 1) * 5, :],
                                           h1psB[:, :, :])

                    # LN#1: mean-center per query (rsqrt folded out), relu
                    h1rs = []
                    for hi, h1h in enumerate((h1A, h1B)):
                        h1d = psm.tile([CG, 75], F32, tag="h1d")
                        nc.vector.tensor_reduce(h1d[:, :].unsqueeze(2),
                                                h1h[:, :, :], AX.X, AL.add)
                        s1p = psms.tile([128, QI], F32, tag="pmm")
                        nc.tensor.matmul(s1p[:2, :75], e2[:, :], h1d[:, :],
                                         start=True, stop=True)
                        mu1 = psm.tile([2, 75], F32, tag="mu1")
                        nc.any.tensor_scalar(mu1[:, :], s1p[:2, :75],
                                             1.0 / 2048.0, None, AL.mult)
                        m1e = psms.tile([128, QI], F32, tag="pmm")
                        nc.tensor.matmul(m1e[:CG, :75], e2t[:, :], mu1[:, :],
                                         start=True, stop=True)
                        mu1e = psm.tile([CG, 75], F32, tag="mu1e")
                        nc.any.tensor_copy(mu1e[:, :], m1e[:CG, :75])
                        for hq in range(2):
                            q4 = hi * 2 + hq
                            pb_ = hq * PIN
                            h1r = ph.tile([PIN, 75, CG], BF16,
                                          tag=f"h1rq{q4}")
                            nc.vector.tensor_tensor(
                                h1r[:, :, :], h1h[pb_:pb_ + PIN, :, :],
                                mu1e[pb_:pb_ + PIN, :].unsqueeze(
                                    2).to_broadcast([PIN, 75, CG]),
                                AL.subtract)
                            nc.any.tensor_scalar(
                                h1r[:, :, :].rearrange("p a b -> p (a b)"),
                                h1r[:, :, :].rearrange("p a b -> p (a b)"),
                                0.0, None, AL.max)
                            h1rs.append(h1r)

                    # ---- mix2: h2[q] = S_q @ h1r_q -> h2sb [128 o, 300, 64]
                    h2sb = ph.tile([128, QI, CG], BF16, tag="h2sb")
                    for qcb in range(15):
                        blk = pdram[qoff + qcb * 20:qoff + (qcb + 1) * 20,
                                    4096:8192].rearrange(
                            "(i q) (p o) -> q p i o", i=5, p=PIN)
                        sis = []
                        for q4 in range(4):
                            si = pms.tile([PIN, 5, 128], BF16,
                                          tag=f"siq{q4}")
                            nc.sync.dma_start(si[:, :, :], blk[q4])
                            sis.append(si)
                        for jj in range(4):
                            h2ps = psh2.tile([128, 5, CG], F32, tag="h2ps")
                            for j in range(5):
                                i20 = jj * 5 + j
                                i5 = i20 // 4
                                qc = qcb * 5 + i5
                                q4 = i20 % 4
                                nc.tensor.matmul(
                                    h2ps[:, j, :],
                                    sis[q4][:, i5, :],
                                    h1rs[q4][:, qc, :],
                                    start=True, stop=True)
                            nc.any.tensor_copy(
                                h2sb[:, qcb * 20 + jj * 5:
                                     qcb * 20 + (jj + 1) * 5, :],
                                h2ps[:, :, :])

                    # LN#2 stats (over o,d per query)
                    h2d = psm.tile([128, QI], F32, tag="h2d")
                    nc.vector.tensor_reduce(h2d[:, :].unsqueeze(2),
                                            h2sb[:, :, :], AX.X, AL.add)
                    sqd2 = psm.tile([128, QI], F32, tag="sqd2")
                    for kk in range(12):
                        sl = slice(kk * 25, (kk + 1) * 25)
                        sq2 = psqp.tile([128, 25 * CG], F32, tag="sq")
                        nc.scalar.activation(
                            sq2[:, :],
                            h2sb[:, sl, :].rearrange("p a b -> p (a b)"),
                            AF.Square)
                        nc.vector.tensor_reduce(
                            sqd2[:, sl].unsqueeze(2),
                            sq2[:, :].rearrange("p (a b) -> p a b", b=CG),
                            AX.X, AL.add)
                    s1q = psms.tile([128, QI], F32, tag="pmm")
                    nc.tensor.matmul(s1q[:1, :], onesc_f[:, :], h2d[:, :],
                                     start=True, stop=True)
                    s2q = psms.tile([128, QI], F32, tag="pmm")
                    nc.tensor.matmul(s2q[:1, :], onesc_f[:, :], sqd2[:, :],
                                     start=True, stop=True)
                    mu2 = psm.tile([1, QI], F32, tag="mu2")
                    nc.any.tensor_scalar(mu2[:, :], s1q[:1, :], 1.0 / 8192.0,
                                         None, AL.mult)
                    ex2 = psm.tile([1, QI], F32, tag="ex2")
                    nc.any.tensor_scalar(ex2[:, :], s2q[:1, :], 1.0 / 8192.0,
                                         None, AL.mult)
                    var2 = psm.tile([1, QI], F32, tag="var2")
                    nc.vector.tensor_tensor(var2[:, :], mu2[:, :], mu2[:, :],
                                            AL.mult)
                    nc.vector.tensor_tensor(var2[:, :], ex2[:, :], var2[:, :],
                                            AL.subtract)
                    r2 = psm.tile([1, QI], F32, tag="r2")
                    nc.any.tensor_scalar(var2[:, :], var2[:, :], 1e-5,
                                         None, AL.add)
                    nc.scalar.activation(r2[:, :], var2[:, :], AF.Sqrt)
                    nc.vector.reciprocal(r2[:, :], r2[:, :])
                    m2e = psms.tile([128, QI], F32, tag="pmm")
                    nc.tensor.matmul(m2e[:, :], ones_f[:, :], mu2[:, :],
                                     start=True, stop=True)
                    mu2e = psm.tile([128, QI], F32, tag="mu2e")
                    nc.any.tensor_copy(mu2e[:, :], m2e[:, :])
                    r2ep = psms.tile([128, QI], F32, tag="pmm")
                    nc.tensor.matmul(r2ep[:, :], ones_f[:, :], r2[:, :],
                                     start=True, stop=True)
                    r2e = psm.tile([128, QI], F32, tag="r2e")
                    nc.any.tensor_copy(r2e[:, :], r2ep[:, :])

                    # h2r = relu(h2 - mu2) in place
                    nc.vector.tensor_tensor(
                        h2sb[:, :, :], h2sb[:, :, :],
                        mu2e[:, :].unsqueeze(2).to_broadcast([128, QI, CG]),
                        AL.subtract)
                    nc.any.tensor_scalar(
                        h2sb[:, :, :].rearrange("p a b -> p (a b)"),
                        h2sb[:, :, :].rearrange("p a b -> p (a b)"),
                        0.0, None, AL.max)

                    # ---- Phase D: projection, contract over o with PSUM
                    # accumulation over d; h2sb stays in SBUF ----
                    pr0 = pso.tile([128, QI], F32, tag="pr0")
                    pr1 = pso.tile([128, QI], F32, tag="pr1")
                    prps = [pr0, pr1]
                    for d in range(CG):
                        ow = pdw.tile([128, D], BF16, tag="ow")
                        nc.sync.dma_start(ow[:, :], opw_dec[d])
                        for dh in range(2):
                            nc.tensor.matmul(
                                prps[dh][:, :],
                                ow[:, dh * 128:(dh + 1) * 128],
                                h2sb[:, :, d],
                                start=(d == 0), stop=(d == CG - 1))
                    for dh in range(2):
                        osb = pout.tile([128, QI], BF16, tag="osb")
                        nc.vector.tensor_tensor(
                            osb[:, :], prps[dh][:, :], r2e[:, :], AL.mult)
                        nc.sync.dma_start(
                            out_ext[dh, :, qoff:qoff + QI], osb[:, :])
    nc.compile()
    _CACHE["nc"] = nc
    return nc


def _host_prep(feats, query_feat, query_roi, off_w, off_b, pg_w, pg_b, op_w):
    """Vectorized numpy: addressing metadata + per-core input tensors."""
    qf = query_feat.astype(np.float32)
    offset = (qf @ off_w + off_b).reshape(B, N, G * PIN, 3)
    roi_cc = query_roi[..., :2]
    scale = 2.0 ** query_roi[..., 2:3]
    ratio = 2.0 ** np.concatenate(
        [query_roi[..., 3:4] * -0.5, query_roi[..., 3:4] * 0.5], axis=-1)
    roi_wh = scale * ratio
    sample_xy = roi_cc[:, :, None, :] + offset[..., :2] * roi_wh[:, :, None, :]
    sample_z = query_roi[..., 2:3] + offset[..., 2]
    lvl = np.arange(4, dtype=np.float32)
    logits = -((sample_z - MAP_STRIDE)[..., None] - lvl) ** 2 / TAU
    logits -= logits.max(-1, keepdims=True)
    e = np.exp(logits)
    lw = (e / e.sum(-1, keepdims=True)).astype(np.float32)  # [B,N,G*PIN,4]
    sx = sample_xy[..., 0]                                  # [B,N,G*PIN]
    sy = sample_xy[..., 1]

    # per (lvl, ycorner): pair base index + 2 slot weights, [B, N, G*PIN]
    idx_all = np.zeros((4, 2, B, N, G * PIN), np.int16)
    w_all = np.zeros((4, 2, 2, B, N, G * PIN), np.float32)
    for li, ((H, W), stride) in enumerate(zip(SIZES, STRIDES)):
        px = sx / stride - 0.5
        py = sy / stride - 0.5
        x0 = np.floor(px)
        y0 = np.floor(py)
        wx1 = (px - x0).astype(np.float32)
        wy1 = (py - y0).astype(np.float32)
        vx0 = (x0 >= 0) & (x0 <= W - 1)
        vx1 = (x0 + 1 >= 0) & (x0 + 1 <= W - 1)
        xc = np.clip(x0, -1, W - 1)
        for dy in range(2):
            yd = y0 + dy
            vy = (yd >= 0) & (yd <= H - 1)
            yc = np.clip(yd, 0, H - 1)
            wy = (wy1 if dy else 1.0 - wy1) * lw[..., li]
            idx_all[li, dy] = (yc * W + xc + 1).astype(np.int16)
            w_all[li, dy, 0] = (1.0 - wx1) * wy * (vx0 & vy)
            w_all[li, dy, 1] = wx1 * wy * (vx1 & vy)

    ew = np.zeros((64, 2), np.float32)
    for p in range(64):
        ew[p, p // 32] = 1.0

    # S column order: device col 4096 + p*128 + o holds S[o, p]
    p_idx = np.repeat(np.arange(PIN), 128)
    o_idx = np.tile(np.arange(128), PIN)
    scol = 4096 + o_idx * PIN + p_idx
    cols = np.concatenate([np.arange(4096), scol])

    qft_pk, s_qft = _pack10(
        np.ascontiguousarray(qf.reshape(QT, D).T.reshape(2, 128, QT)),
        plane_axis=1)

    in_maps = []
    for g in range(G):
        # features: flat [pad, img0 lvls, img1..., pad] x 64ch, 10-bit packed
        fparts = [np.zeros((1, CG), np.float32)]
        for b in range(B):
            for li, (H, W) in enumerate(SIZES):
                f = feats[li][b, g * CG:(g + 1) * CG]      # [64, H, W]
                fparts.append(f.reshape(CG, H * W).T)
        fcat = np.concatenate(fparts, axis=0)
        pad_tail = PIXPAD - fcat.shape[0]
        fcat = np.concatenate(
            [fcat, np.zeros((pad_tail, CG), np.float32)], axis=0)
        fb_pk, s_fb = _pack9(fcat.reshape(128, FLATC), plane_axis=0)

        idx_cols = np.empty((16, NGRP * 600), np.int16)
        cw_cols = np.empty((128, NGRP * 150), np.float32)
        for b in range(B):
            for li in range(4):
                for dy in range(2):
                    ci = b * 8 + li * 2 + dy
                    ia = idx_all[li, dy, b, :, g * PIN:(g + 1) * PIN]
                    idx_cols[:, ci * 600:(ci + 1) * 600] = \
                        ia.reshape(NSMP).reshape(600, 16).T
                    wp = np.stack(
                        [w_all[li, dy, s, b, :, g * PIN:(g + 1) * PIN]
                         .reshape(NSMP) for s in range(2)], axis=-1)
                    cw_cols[:, ci * 150:(ci + 1) * 150] = \
                        wp.reshape(75, 128, 2).transpose(1, 0, 2).reshape(
                            128, 150)
        cw_pk, s_cw = _pack10(cw_cols, plane_axis=0)

        pgw_c = pg_w[:, g * 8192:(g + 1) * 8192][:, cols].astype(
            np.float32).reshape(2, 128, 8192)
        pgw_pk, s_pgw = _pack9(pgw_c, plane_axis=1)
        pgb_c = (pg_b[g * 8192:(g + 1) * 8192][cols] * (s_pgw * s_qft)
                 ).astype(BF)[None, :]
        scl_c = np.stack([
            np.full(128, 1.0 / (s_pgw * s_qft), np.float32),
            np.full(128, 1.0 / (s_cw * s_fb), np.float32)], axis=1)
        opw_c = op_w[g * 8192:(g + 1) * 8192, :].reshape(128, CG, D)\
            .transpose(1, 0, 2).astype(np.float32)
        opw_pk, s_opw = _pack9(opw_c, plane_axis=1)
        in_maps.append({
            "fb": fb_pk, "idx": np.ascontiguousarray(idx_cols),
            "cw": cw_pk,
            "pgw": pgw_pk, "pgb": np.ascontiguousarray(pgb_c),
            "qft": qft_pk, "opw": opw_pk, "scl": scl_c,
            "e2": ew, "e2t": np.ascontiguousarray(ew.T),
            "_host_s_opw": s_opw,      # host-only: not a declared input
        })
    return in_maps


def kernel(feat0, feat1, feat2, feat3, query_feat, query_roi,
           off_w, off_b, pg_w, pg_b, op_w, op_b, ln_g, ln_b):
    feats = [np.asarray(f, np.float32) for f in (feat0, feat1, feat2, feat3)]
    query_feat = np.asarray(query_feat, np.float32)
    query_roi = np.asarray(query_roi, np.float32)
    in_maps = _host_prep(feats, query_feat, query_roi,
                         np.asarray(off_w, np.float32),
                         np.asarray(off_b, np.float32),
                         np.asarray(pg_w, np.float32),
                         np.asarray(pg_b, np.float32),
                         np.asarray(op_w, np.float32))
    nc = _build()
    cores = list(range(G))
    if not _CACHE.get("warm"):
        # first call compiles/loads the PJRT executable; run it once so the
        # steady-state call below reflects transfer+execute only
        run_bass_kernel_spmd(nc, in_maps, core_ids=cores)
        _CACHE["warm"] = True
    res = run_bass_kernel_spmd(nc, in_maps, core_ids=cores)
    outs = res.results

    op_b = np.asarray(op_b, np.float32)
    ln_g = np.asarray(ln_g, np.float32)
    ln_b = np.asarray(ln_b, np.float32)
    acc = np.zeros((D, QT), np.float32)
    for g in range(G):
        o = outs[g]
        o = o["out"] if isinstance(o, dict) else o[0]
        acc += np.asarray(o, np.float32).reshape(D, QT) / \
            in_maps[g]["_host_s_opw"]
    h = acc.T.reshape(B, N, D) + query_feat + op_b
    mu = h.mean(-1, keepdims=True)
    var = ((h - mu) ** 2).mean(-1, keepdims=True)
    return (h - mu) / np.sqrt(var + 1e-5) * ln_g + ln_b
